# revision 1
# baseline (speedup 1.0000x reference)
"""Self-contained Trainium2 kernel for nn_FDN_37211596653125.

kernel(**inputs) -> y (32,2,441000) float32.
Host: FDN impulse response (tiny 6x6 solves). Device (8 NeuronCores, raw
Bass): overlap-save FFT convolution, N=131072=256x512 Cooley-Tukey via PE
matmuls, twiddle/spectral products on VectorE, PSUM evacuation on ScalarE.
"""
import sys
sys.path.insert(0, "/opt/trn_rl_repo")
import numpy as np
import concourse.bass as bass
import concourse.mybir as mybir
from concourse.masks import make_identity


SR = 44100
IR_LEN = 44100
T60 = 0.75
D = 6


def _expm(A):
    """Pade-13 scaling-and-squaring matrix exponential (float64)."""
    A = A.astype(np.float64)
    b = [64764752532480000.0, 32382376266240000.0, 7771770303897600.0,
         1187353796428800.0, 129060195264000.0, 10559470521600.0,
         670442572800.0, 33522128640.0, 1323241920.0, 40840800.0,
         960960.0, 16380.0, 182.0, 1.0]
    n = A.shape[0]
    nrm = np.linalg.norm(A, 1)
    theta13 = 5.371920351148152
    s = max(0, int(np.ceil(np.log2(max(nrm / theta13, 1e-300)))))
    if nrm <= theta13:
        s = 0
    A = A / (2.0 ** s)
    I = np.eye(n)
    A2 = A @ A
    A4 = A2 @ A2
    A6 = A2 @ A4
    U = A @ (A6 @ (b[13] * A6 + b[11] * A4 + b[9] * A2)
             + b[7] * A6 + b[5] * A4 + b[3] * A2 + b[1] * I)
    V = (A6 @ (b[12] * A6 + b[10] * A4 + b[8] * A2)
         + b[6] * A6 + b[4] * A4 + b[2] * A2 + b[0] * I)
    R = np.linalg.solve(V - U, V + U)
    for _ in range(s):
        R = R @ R
    return R


def fdn_ir(b, c, U_raw, gamma_raw, delays):
    """Build (2,2,IR_LEN) float32 FDN impulse response."""
    delays_f = delays.astype(np.float32)
    F_ = IR_LEN // 2 + 1
    gamma_max = np.float32(10.0) ** (np.float32(-60.0 / SR / T60 / 20.0)
                                     * delays_f)               # (D,)
    gamma = (1.0 / (1.0 + np.exp(-gamma_raw.astype(np.float32)))) * gamma_max  # (S,D)
    S = gamma.shape[0]
    pos = np.arange(F_, dtype=np.float32) * np.float32((S - 1) / (F_ - 1))
    lo = np.clip(np.floor(pos).astype(np.int32), 0, S - 2)
    frac = (pos - lo.astype(np.float32))[:, None]
    g = gamma[lo] * (1.0 - frac) + gamma[lo + 1] * frac         # (F,D) fp32

    tri = np.triu(U_raw.astype(np.float64), 1)
    U = _expm(tri - tri.T).astype(np.float32)                   # (D,D)

    A = U[None, :, :] * g[:, None, :]                           # (F,D,D)
    freqs = (np.arange(F_, dtype=np.float32) / np.float32(IR_LEN)
             * np.float32(2.0 * np.pi))
    phase = freqs[:, None] * delays_f[None, :]                  # fp32 like ref
    invD = np.exp(1j * phase.astype(np.float32)).astype(np.complex64)  # (F,D)
    eye = np.eye(D, dtype=np.complex64)
    M = invD[:, :, None] * eye[None] - A.astype(np.complex64)   # (F,D,D)
    b_c = np.broadcast_to(b.astype(np.complex64), (F_, D, 2))
    X = np.linalg.solve(M, b_c)                                 # (F,D,2)
    H = np.einsum('oi,fik->fok', c.astype(np.complex64), X)     # (F,2,2)
    h = np.fft.irfft(H.transpose(1, 2, 0), n=IR_LEN, axis=-1)   # (2,2,L)
    return h.astype(np.float32)





FP = mybir.dt.float32

N = 131072
N1, N2 = 256, 512
K_IR = 44100
L_HOP = N - K_IR + 1            # 86973
T_SIG = 441000
NBLK = 6
PAD_PRE = K_IR - 1              # 44099
PAD_LEN = (NBLK - 1) * L_HOP + N  # 565937
NB = 4                          # batches per core
NIT = NB * NBLK                 # 24

# ---- per-iteration semaphore increment schedules -------------------------
PE_PER = 40   # A0=1, T0=8, C0=4, A1=1, T1=8, C1=4, Ap=4, Tp=8, Cp=2
DVE_PER = 16  # tw0=2, fdl0=4, tw1=2, fdl1=4, twp=4
ACT_PER = 28  # cT0=8, cT1=8, cS=8, cy=4


def pe_m(it):
    b = PE_PER * it
    return dict(A0=b + 1, T0=[b + 1 + k for k in range(1, 9)],
                C0=[b + 9 + k for k in range(1, 5)], A1=b + 14,
                T1=[b + 14 + k for k in range(1, 9)],
                C1=[b + 22 + k for k in range(1, 5)],
                Ap=[b + 26 + k for k in range(1, 5)],
                Tp=[b + 30 + k for k in range(1, 9)],
                Cp=[b + 38 + k for k in range(1, 3)])


def dve_m(it):
    b = DVE_PER * it
    return dict(tw0=[b + k for k in range(1, 3)],
                fdl0=[b + 2 + k for k in range(1, 5)],
                tw1=[b + 6 + k for k in range(1, 3)],
                fdl1=[b + 8 + k for k in range(1, 5)],
                twp=[b + 12 + k for k in range(1, 5)])


def act_m(it):
    b = ACT_PER * it
    return dict(cT0=[b + k for k in range(1, 9)],
                cT1=[b + 8 + k for k in range(1, 9)],
                cS=[b + 16 + k for k in range(1, 9)],
                cy=[b + 24 + k for k in range(1, 5)])


def out_ranges(j):
    """DMA row ranges for valid region of block j.
    Returns list of (n1c, row_lo, row_hi, col_lo, col_hi, dest_off) where
    rows are local to chunk n1c, dest_off is offset into y[b,ch] flat."""
    valid = min(L_HOP, T_SIG - j * L_HOP)
    p0, p1 = PAD_PRE, PAD_PRE + valid
    res = []
    for n1c in range(2):
        base = 128 * n1c
        segs = []
        for R in range(base, base + 128):
            r0, r1 = 512 * R, 512 * R + 512
            s, e = max(r0, p0), min(r1, p1)
            if s >= e:
                continue
            segs.append((R, s - r0, e - r0, s))
        # merge full rows into spans
        i = 0
        while i < len(segs):
            R, c0, c1, dst = segs[i]
            if c0 == 0 and c1 == 512:
                k = i
                while (k + 1 < len(segs) and segs[k + 1][1] == 0
                       and segs[k + 1][2] == 512):
                    k += 1
                res.append((n1c, segs[i][0] - base, segs[k][0] - base + 1,
                            0, 512, dst - p0 + j * L_HOP))
                i = k + 1
            else:
                res.append((n1c, R - base, R - base + 1, c0, c1,
                            dst - p0 + j * L_HOP))
                i += 1
    return res


def n_out_dmas(j):
    return 2 * len(out_ranges(j))   # x2 planes


def build_nc():
    nc = bass.Bass()
    xp_in = nc.declare_dram_parameter("xp", [NB, 2, PAD_LEN], FP, isOutput=False)
    w256_in = nc.declare_dram_parameter("w256", [128, 12 * 128], FP, isOutput=False)
    w512_in = nc.declare_dram_parameter("w512", [128, 48 * 128], FP, isOutput=False)
    tw_in = nc.declare_dram_parameter("tw", [128, 4 * N2], FP, isOutput=False)
    twi_in = nc.declare_dram_parameter("twi", [128, 8 * 256], FP, isOutput=False)
    g_in = nc.declare_dram_parameter("g", [128, 16 * 256], FP, isOutput=False)
    y_out = nc.declare_dram_parameter("y", [NB, 2, T_SIG], FP, isOutput=True)

    # dma counter schedule: prologue tables = 6 DMAs (incl identity? no)
    NTAB = 5
    dma_v = [16 * NTAB]  # value after prologue
    ld_after_ch0 = {}
    ld_after = {}
    out_after = {}
    v = 16 * NTAB
    for it in range(NIT):
        v += 32; ld_after_ch0[it] = v
        v += 32; ld_after[it] = v
        j = it % NBLK
        v += 16 * n_out_dmas(j); out_after[it] = v

    from contextlib import ExitStack
    es = ExitStack()
    with es:
        w256 = es.enter_context(nc.sbuf_tensor([128, 12 * 128], FP))
        w512 = es.enter_context(nc.sbuf_tensor([128, 48 * 128], FP))
        tw = es.enter_context(nc.sbuf_tensor([128, 4 * N2], FP))
        twi = es.enter_context(nc.sbuf_tensor([128, 8 * 256], FP))
        gtab = es.enter_context(nc.sbuf_tensor([128, 16 * 256], FP))
        ident = es.enter_context(nc.sbuf_tensor([128, 128], FP))
        m1 = es.enter_context(nc.sbuf_tensor([128, 2 * 2048], FP))
        t2 = es.enter_context(nc.sbuf_tensor([128, 4 * N2], FP))
        t2t = es.enter_context(nc.sbuf_tensor([128, 8 * 256], FP))
        ymat = es.enter_context(nc.sbuf_tensor([128, 8 * 256], FP))
        s2 = es.enter_context(nc.sbuf_tensor([128, 8 * 256], FP))
        s2t = es.enter_context(nc.sbuf_tensor([128, 4 * N2], FP))
        ysb = es.enter_context(nc.sbuf_tensor([128, 2 * 2048], FP))
        dvetmp = es.enter_context(nc.sbuf_tensor([128, 2 * N2], FP))
        pbanks = [es.enter_context(nc.psum_tensor(f"pb{i}", [128, N2], FP)) for i in range(8)]
        (pb0, pb1, pb2, pb3, pb4, pb5, pb6, pb7) = pbanks
        s_dma = es.enter_context(nc.semaphore("s_dma"))
        s_pe = es.enter_context(nc.semaphore("s_pe"))
        s_dve = es.enter_context(nc.semaphore("s_dve"))
        s_act = es.enter_context(nc.semaphore("s_act"))
        s_gp = es.enter_context(nc.semaphore("s_gp"))
        block = es.enter_context(nc.Block())
        psA = [pb0, pb1, pb2, pb3]   # re0,im0,re1,im1 (fwd) | n2c (inv, re|im packed)
        psC = [pb4, pb5, pb6, pb7]   # k2c (fwd, re|im packed) | n1c*2+plane (inv)
        psT = [pb0, pb1]             # transpose slots (alternate)

        # table slice helpers
        def w256c(n1c, k1c, plane):  # plane 0=re,1=im,2=imn
            o = (plane * 4 + n1c * 2 + k1c) * 128
            return w256[:, o:o + 128]

        def w512c(n2c, k2c, plane):
            o = (plane * 16 + n2c * 4 + k2c) * 128
            return w512[:, o:o + 128]

        def twc(k1c, plane):
            return tw[:, (plane * 2 + k1c) * N2:(plane * 2 + k1c + 1) * N2]

        def twic(n2c, plane):
            return twi[:, (plane * 4 + n2c) * 256:(plane * 4 + n2c + 1) * 256]

        def gc(i, k2c, plane):
            o = ((i * 2 + plane) * 4 + k2c) * 256
            return gtab[:, o:o + 256]

        def t2c(k1c, plane):
            return t2[:, (plane * 2 + k1c) * N2:(plane * 2 + k1c + 1) * N2]

        def t2tc(n2c, plane):
            return t2t[:, (plane * 4 + n2c) * 256:(plane * 4 + n2c + 1) * 256]

        def ymc(k2c, plane):
            return ymat[:, (plane * 4 + k2c) * 256:(plane * 4 + k2c + 1) * 256]

        def s2c(n2c, plane):
            return s2[:, (plane * 4 + n2c) * 256:(plane * 4 + n2c + 1) * 256]

        def s2tc(k1c, plane):
            return s2t[:, (plane * 2 + k1c) * N2:(plane * 2 + k1c + 1) * N2]

        @block.gpsimd
        def _(gpsimd):
            make_identity(nc, ident[:, :])
            gpsimd.sem_inc(s_gp, 1)

        def emit_outs(sync, it):
            b, j = divmod(it, NBLK)
            buf = it % 2
            sync.wait_ge(s_act, act_m(it)["cy"][3])
            for plane in range(2):          # 0=re->ch0, 1=im->ch1
                for (n1c, rlo, rhi, clo, chi, doff) in out_ranges(j):
                    o_ = buf * 2048 + (n1c * 2 + plane) * N2
                    src = ysb[rlo:rhi, o_ + clo:o_ + chi]
                    cnt = (rhi - rlo) * (chi - clo)
                    dst = y_out[b, plane, doff:doff + cnt]
                    dst = dst.rearrange("(p f) -> p f", f=chi - clo)
                    sync.dma_start(dst, src).then_inc(s_dma, 16)

        @block.sync
        def _(sync):
            sync.dma_start(w256[:, :], w256_in[:, :]).then_inc(s_dma, 16)
            sync.dma_start(w512[:, :], w512_in[:, :]).then_inc(s_dma, 16)
            sync.dma_start(tw[:, :], tw_in[:, :]).then_inc(s_dma, 16)
            sync.dma_start(twi[:, :], twi_in[:, :]).then_inc(s_dma, 16)
            sync.dma_start(gtab[:, :], g_in[:, :]).then_inc(s_dma, 16)
            for it in range(NIT):
                b, j = divmod(it, NBLK)
                buf = it % 2
                if it >= 2:
                    sync.wait_ge(s_pe, pe_m(it - 2)["A1"])
                for ch in range(2):
                    src = xp_in[b, ch, j * L_HOP: j * L_HOP + N]
                    src = src.rearrange("(p f) -> p f", f=N2)
                    for n1c in range(2):
                        o_ = buf * 2048 + (ch * 2 + n1c) * N2
                        sync.dma_start(
                            m1[:, o_:o_ + N2],
                            src[128 * n1c:128 * (n1c + 1), :],
                        ).then_inc(s_dma, 16)
                # outputs of previous iteration
                if it >= 1:
                    emit_outs(sync, it - 1)
            emit_outs(sync, NIT - 1)

        @block.tensor
        def _(tensor):
            tensor.wait_ge(s_gp, 1)
            tensor.wait_ge(s_dma, 16 * 5)
            for it in range(NIT):
                pm, dm, am = pe_m(it), dve_m(it), act_m(it)
                buf = it % 2

                def m1c(ch, n1c):
                    o_ = buf * 2048 + (ch * 2 + n1c) * N2
                    return m1[:, o_:o_ + N2]

                # ---------------- forward FFT per channel ----------------
                for ch in range(2):
                    # stage A: T1[k1c] (re->psA[2*k1c?] layout re0,im0,re1,im1)
                    if ch == 0:
                        tensor.wait_ge(s_dma, ld_after_ch0[it])
                        if it >= 1:
                            tensor.wait_ge(s_dve, dve_m(it - 1)["twp"][3])
                            tensor.wait_ge(s_act, act_m(it - 1)["cS"][7])
                    else:
                        tensor.wait_ge(s_dma, ld_after[it])
                        tensor.wait_ge(s_dve, dm["tw0"][1])
                        tensor.wait_ge(s_act, am["cT0"][7])
                    last = None
                    for k1c in range(2):
                        pre, pim = psA[2 * k1c], psA[2 * k1c + 1]
                        for n1c in range(2):
                            nc.tensor.matmul(pre[:, :], w256c(n1c, k1c, 0),
                                             m1c(ch, n1c), start=(n1c == 0),
                                             stop=(n1c == 1))
                        for n1c in range(2):
                            last = nc.tensor.matmul(
                                pim[:, :], w256c(n1c, k1c, 1), m1c(ch, n1c),
                                start=(n1c == 0), stop=(n1c == 1))
                    last.then_inc(s_pe, 1)      # A done
                    # transposes: pairs (n2c,k1c-combined): 8 pairs of (re,im)
                    twk = dm["tw0"] if ch == 0 else dm["tw1"]
                    cTk = am["cT0"] if ch == 0 else am["cT1"]
                    p = 0
                    for n2c in range(4):
                        for k1c in range(2):
                            tensor.wait_ge(s_dve, twk[k1c])
                            if p >= 2:
                                tensor.wait_ge(s_act, cTk[p - 2])
                            slot = psT[p % 2]
                            nc.tensor.transpose(
                                slot[:, 0:128],
                                t2c(k1c, 0)[:, n2c * 128:(n2c + 1) * 128],
                                ident[:, :])
                            nc.tensor.transpose(
                                slot[:, 128:256],
                                t2c(k1c, 1)[:, n2c * 128:(n2c + 1) * 128],
                                ident[:, :]).then_inc(s_pe, 1)
                            p += 1
                    # stage C: X[k2c] re|im packed into psC[k2c]
                    tensor.wait_ge(s_act, cTk[7])
                    if ch == 0 and it >= 1:
                        tensor.wait_ge(s_act, act_m(it - 1)["cy"][3])
                    if ch == 1:
                        tensor.wait_ge(s_dve, dm["fdl0"][3])
                    for k2c in range(4):
                        pre = psC[k2c][:, 0:256]
                        pim = psC[k2c][:, 256:512]
                        seq = []
                        for n2c in range(4):
                            seq.append((pre, w512c(n2c, k2c, 0), t2tc(n2c, 0)))
                        for n2c in range(4):
                            seq.append((pre, w512c(n2c, k2c, 2), t2tc(n2c, 1)))
                        for i_, (dst, w_, r_) in enumerate(seq):
                            nc.tensor.matmul(dst, w_, r_, start=(i_ == 0),
                                             stop=(i_ == 7))
                        seq = []
                        for n2c in range(4):
                            seq.append((pim, w512c(n2c, k2c, 0), t2tc(n2c, 1)))
                        for n2c in range(4):
                            seq.append((pim, w512c(n2c, k2c, 1), t2tc(n2c, 0)))
                        for i_, (dst, w_, r_) in enumerate(seq):
                            mm = nc.tensor.matmul(dst, w_, r_, start=(i_ == 0),
                                                  stop=(i_ == 7))
                        mm.then_inc(s_pe, 1)
                # ---------------- inverse ----------------
                # stage A': S[n2c] = V512 @ Ymat ; V512: re=w512re, im=w512imn, imn=w512im
                tensor.wait_ge(s_dve, dm["fdl1"][3])
                tensor.wait_ge(s_dve, dm["tw1"][1])
                tensor.wait_ge(s_act, am["cT1"][7])
                for n2c in range(4):
                    pre = psA[n2c][:, 0:256]
                    pim = psA[n2c][:, 256:512]
                    seq = []
                    for k2c in range(4):
                        seq.append((pre, w512c(k2c, n2c, 0), ymc(k2c, 0)))
                    for k2c in range(4):
                        seq.append((pre, w512c(k2c, n2c, 1), ymc(k2c, 1)))
                    for i_, (dst, w_, r_) in enumerate(seq):
                        nc.tensor.matmul(dst, w_, r_, start=(i_ == 0),
                                         stop=(i_ == 7))
                    seq = []
                    for k2c in range(4):
                        seq.append((pim, w512c(k2c, n2c, 0), ymc(k2c, 1)))
                    for k2c in range(4):
                        seq.append((pim, w512c(k2c, n2c, 2), ymc(k2c, 0)))
                    for i_, (dst, w_, r_) in enumerate(seq):
                        mm = nc.tensor.matmul(dst, w_, r_, start=(i_ == 0),
                                              stop=(i_ == 7))
                    mm.then_inc(s_pe, 1)
                # transposes of S2 -> psT -> (ACT) s2t
                p = 0
                for n2c in range(4):
                    for k1c in range(2):
                        tensor.wait_ge(s_dve, dm["twp"][max(n2c, p % 2)])
                        if p >= 2:
                            tensor.wait_ge(s_act, am["cS"][p - 2])
                        slot = psT[p % 2]
                        nc.tensor.transpose(
                            slot[:, 0:128],
                            s2c(n2c, 0)[:, k1c * 128:(k1c + 1) * 128],
                            ident[:, :])
                        nc.tensor.transpose(
                            slot[:, 128:256],
                            s2c(n2c, 1)[:, k1c * 128:(k1c + 1) * 128],
                            ident[:, :]).then_inc(s_pe, 1)
                        p += 1
                # stage C': y[n1c] = V256 @ S2t ; V256: re=w256re, im=w256imn->plane2? V=conj
                tensor.wait_ge(s_act, am["cS"][7])
                for n1c in range(2):
                    pre = psC[2 * n1c]      # full 512 wide (re plane)
                    pim = psC[2 * n1c + 1]
                    seq = []
                    for k1c in range(2):
                        seq.append((pre, w256c(k1c, n1c, 0), s2tc(k1c, 0)))
                    for k1c in range(2):
                        seq.append((pre, w256c(k1c, n1c, 1), s2tc(k1c, 1)))
                    for i_, (dst, w_, r_) in enumerate(seq):
                        nc.tensor.matmul(dst[:, :], w_, r_, start=(i_ == 0),
                                         stop=(i_ == 3))
                    seq = []
                    for k1c in range(2):
                        seq.append((pim, w256c(k1c, n1c, 0), s2tc(k1c, 1)))
                    for k1c in range(2):
                        seq.append((pim, w256c(k1c, n1c, 2), s2tc(k1c, 0)))
                    for i_, (dst, w_, r_) in enumerate(seq):
                        mm = nc.tensor.matmul(dst[:, :], w_, r_, start=(i_ == 0),
                                              stop=(i_ == 3))
                    mm.then_inc(s_pe, 1)

        @block.vector
        def _(vector):
            for it in range(NIT):
                pm, dm, am = pe_m(it), dve_m(it), act_m(it)

                def cmul_psum(dst_re, dst_im, src_re, src_im, twr, twi_,
                              tmp1, tmp2):
                    nc.vector.tensor_mul(dst_re, src_re, twr)
                    nc.vector.tensor_mul(tmp1, src_im, twi_)
                    nc.vector.tensor_sub(dst_re, dst_re, tmp1)
                    nc.vector.tensor_mul(dst_im, src_re, twi_)
                    nc.vector.tensor_mul(tmp2, src_im, twr)
                    return nc.vector.tensor_add(dst_im, dst_im, tmp2)

                for ch in range(2):
                    Ad = pm["A0"] if ch == 0 else pm["A1"]
                    twk = dm["tw0"] if ch == 0 else dm["tw1"]
                    vector.wait_ge(s_pe, Ad)
                    for k1c in range(2):
                        cmul_psum(t2c(k1c, 0), t2c(k1c, 1),
                                  psA[2 * k1c][:, :], psA[2 * k1c + 1][:, :],
                                  twc(k1c, 0), twc(k1c, 1),
                                  dvetmp[:, 0:N2], dvetmp[:, N2:2 * N2]
                                  ).then_inc(s_dve, 1)
                    # FDL
                    Ck = pm["C0"] if ch == 0 else pm["C1"]
                    for k2c in range(4):
                        vector.wait_ge(s_pe, Ck[k2c])
                        xre = psC[k2c][:, 0:256]
                        xim = psC[k2c][:, 256:512]
                        t1 = dvetmp[:, 0:256]
                        if ch == 0:
                            nc.vector.tensor_mul(ymc(k2c, 0), xre, gc(0, k2c, 0))
                            nc.vector.tensor_mul(t1, xim, gc(0, k2c, 1))
                            nc.vector.tensor_sub(ymc(k2c, 0), ymc(k2c, 0), t1)
                            nc.vector.tensor_mul(ymc(k2c, 1), xre, gc(0, k2c, 1))
                            nc.vector.tensor_mul(t1, xim, gc(0, k2c, 0))
                            nc.vector.tensor_add(ymc(k2c, 1), ymc(k2c, 1), t1
                                                 ).then_inc(s_dve, 1)
                        else:
                            nc.vector.tensor_mul(t1, xre, gc(1, k2c, 0))
                            nc.vector.tensor_add(ymc(k2c, 0), ymc(k2c, 0), t1)
                            nc.vector.tensor_mul(t1, xim, gc(1, k2c, 1))
                            nc.vector.tensor_sub(ymc(k2c, 0), ymc(k2c, 0), t1)
                            nc.vector.tensor_mul(t1, xre, gc(1, k2c, 1))
                            nc.vector.tensor_add(ymc(k2c, 1), ymc(k2c, 1), t1)
                            nc.vector.tensor_mul(t1, xim, gc(1, k2c, 0))
                            nc.vector.tensor_add(ymc(k2c, 1), ymc(k2c, 1), t1
                                                 ).then_inc(s_dve, 1)
                # inverse twiddle: S2[n2c] = S[n2c] * TWI[n2c]
                for n2c in range(4):
                    vector.wait_ge(s_pe, pm["Ap"][n2c])
                    cmul_psum(s2c(n2c, 0), s2c(n2c, 1),
                              psA[n2c][:, 0:256], psA[n2c][:, 256:512],
                              twic(n2c, 0), twic(n2c, 1),
                              dvetmp[:, 0:256], dvetmp[:, 256:512]
                              ).then_inc(s_dve, 1)

        @block.scalar
        def _(scalar):
            for it in range(NIT):
                pm, am = pe_m(it), act_m(it)
                buf = it % 2
                for ch in range(2):
                    Tk = pm["T0"] if ch == 0 else pm["T1"]
                    p = 0
                    for n2c in range(4):
                        for k1c in range(2):
                            scalar.wait_ge(s_pe, Tk[p])
                            slot = psT[p % 2]
                            nc.scalar.copy(
                                t2tc(n2c, 0)[:, k1c * 128:(k1c + 1) * 128],
                                slot[:, 0:128])
                            nc.scalar.copy(
                                t2tc(n2c, 1)[:, k1c * 128:(k1c + 1) * 128],
                                slot[:, 128:256]).then_inc(s_act, 1)
                            p += 1
                # inverse transpose copies
                p = 0
                for n2c in range(4):
                    for k1c in range(2):
                        scalar.wait_ge(s_pe, pm["Tp"][p])
                        slot = psT[p % 2]
                        nc.scalar.copy(
                            s2tc(k1c, 0)[:, n2c * 128:(n2c + 1) * 128],
                            slot[:, 0:128])
                        nc.scalar.copy(
                            s2tc(k1c, 1)[:, n2c * 128:(n2c + 1) * 128],
                            slot[:, 128:256]).then_inc(s_act, 1)
                        p += 1
                # y copies psC -> ysb
                if it >= 2:
                    scalar.wait_ge(s_dma, out_after[it - 2])
                for n1c in range(2):
                    scalar.wait_ge(s_pe, pm["Cp"][n1c])
                    ob = buf * 2048
                    nc.scalar.copy(ysb[:, ob + (n1c * 2 + 0) * N2:ob + (n1c * 2 + 1) * N2],
                                   psC[2 * n1c][:, :])
                    nc.scalar.copy(ysb[:, ob + (n1c * 2 + 1) * N2:ob + (n1c * 2 + 2) * N2],
                                   psC[2 * n1c + 1][:, :]).then_inc(s_act, 1)
                    scalar.sem_inc(s_act, 1)   # cy has 4 slots: 2 per n1c
    return nc


# ---------------------------- host side ----------------------------------

def make_device_tables(h):
    """h: (2,2,K_IR) float32 -> dict of DRAM table arrays (fp32)."""
    def dftm(n, sign):
        k = np.arange(n)
        return np.exp(sign * 2j * np.pi * np.outer(k, k) / n)
    W256 = dftm(N1, -1)
    W512 = dftm(N2, -1)
    k1 = np.arange(N1)
    n2 = np.arange(N2)
    TW = np.exp(-2j * np.pi * np.outer(k1, n2) / N)
    TWI = np.exp(+2j * np.pi * np.outer(n2, k1) / N) / N

    def chunks(M, pr, pc, planes):   # planes: list of 2d arrays [R,C]
        # returns [128, len(planes)*pr*pc*...] col-concatenated in
        # (plane, rowchunk, colchunk) order with 128-col chunks
        cols = []
        for P in planes:
            for a in range(pr):
                for b_ in range(pc):
                    cols.append(P[128 * a:128 * (a + 1),
                                  128 * b_:128 * (b_ + 1)].astype(np.float32))
        return np.ascontiguousarray(np.concatenate(cols, axis=1))

    w256 = chunks(None, 2, 2, [W256.real, W256.imag, -W256.imag])
    w512 = chunks(None, 4, 4, [W512.real, W512.imag, -W512.imag])

    twp = np.concatenate([TW.real[0:128], TW.real[128:256],
                          TW.imag[0:128], TW.imag[128:256]], axis=1)
    twip = np.concatenate([TWI.real[128 * a:128 * (a + 1)] for a in range(4)]
                          + [TWI.imag[128 * a:128 * (a + 1)] for a in range(4)],
                          axis=1)
    hp = np.zeros((2, 2, N), np.float64)
    hp[:, :, :K_IR] = h
    gcols = []
    for i in range(2):
        G = np.fft.fft(hp[0, i]) + 1j * np.fft.fft(hp[1, i])
        Gm = G.reshape(N2, N1)          # [k2, k1]
        for plane in range(2):
            P = Gm.real if plane == 0 else Gm.imag
            for k2c in range(4):
                gcols.append(P[128 * k2c:128 * (k2c + 1), :].astype(np.float32))
    g = np.ascontiguousarray(np.concatenate(gcols, axis=1))
    return dict(w256=w256.astype(np.float32), w512=w512.astype(np.float32),
                tw=twp.astype(np.float32), twi=twip.astype(np.float32),
                g=g.astype(np.float32))


_NC_CACHE = None


def conv_device(x, h):
    """x: (B,2,T) fp32, h: (2,2,K_IR) fp32 -> y: (B,2,T) fp32 via 8 cores."""
    global _NC_CACHE
    from concourse.bass_utils import run_bass_kernel_spmd
    B = x.shape[0]
    assert B == 8 * NB
    xp = np.zeros((B, 2, PAD_LEN), np.float32)
    xp[:, :, PAD_PRE:PAD_PRE + T_SIG] = x
    tabs = make_device_tables(h)
    if _NC_CACHE is None:
        _NC_CACHE = build_nc()
    nc = _NC_CACHE
    in_maps = []
    for c in range(8):
        m = {"xp": xp[NB * c:NB * (c + 1)]}
        m.update(tabs)
        in_maps.append(m)
    res = run_bass_kernel_spmd(nc, in_maps, list(range(8)))
    y = np.concatenate([res.results[c]["y"] for c in range(8)], axis=0)
    return y




def kernel(**inputs):
    """Full FDN: build IR on host, FFT-convolve on 8 NeuronCores."""
    x = np.asarray(inputs["x"], np.float32)
    h = fdn_ir(np.asarray(inputs["b"]), np.asarray(inputs["c"]),
               np.asarray(inputs["U_raw"]), np.asarray(inputs["gamma_raw"]),
               np.asarray(inputs["delays"]))
    y = conv_device(x, h)
    return y.astype(np.float32)



# revision 8
# speedup vs baseline: 1.9827x; 1.9827x over previous
"""Self-contained Trainium2 kernel for nn_FDN_37211596653125.

kernel(**inputs) -> y (32,2,441000) float32.
Host: FDN impulse response (tiny 6x6 solves). Device (8 NeuronCores, raw
Bass): overlap-save FFT convolution, N=131072=256x512 Cooley-Tukey via PE
matmuls, twiddle/spectral products on VectorE, PSUM evacuation on ScalarE.
"""
import sys
sys.path.insert(0, "/opt/trn_rl_repo")
import numpy as np
import concourse.bass as bass
import concourse.mybir as mybir
from concourse.masks import make_identity


SR = 44100
IR_LEN = 44100
T60 = 0.75
D = 6


def _expm(A):
    """Pade-13 scaling-and-squaring matrix exponential (float64)."""
    A = A.astype(np.float64)
    b = [64764752532480000.0, 32382376266240000.0, 7771770303897600.0,
         1187353796428800.0, 129060195264000.0, 10559470521600.0,
         670442572800.0, 33522128640.0, 1323241920.0, 40840800.0,
         960960.0, 16380.0, 182.0, 1.0]
    n = A.shape[0]
    nrm = np.linalg.norm(A, 1)
    theta13 = 5.371920351148152
    s = max(0, int(np.ceil(np.log2(max(nrm / theta13, 1e-300)))))
    if nrm <= theta13:
        s = 0
    A = A / (2.0 ** s)
    I = np.eye(n)
    A2 = A @ A
    A4 = A2 @ A2
    A6 = A2 @ A4
    U = A @ (A6 @ (b[13] * A6 + b[11] * A4 + b[9] * A2)
             + b[7] * A6 + b[5] * A4 + b[3] * A2 + b[1] * I)
    V = (A6 @ (b[12] * A6 + b[10] * A4 + b[8] * A2)
         + b[6] * A6 + b[4] * A4 + b[2] * A2 + b[0] * I)
    R = np.linalg.solve(V - U, V + U)
    for _ in range(s):
        R = R @ R
    return R


def fdn_ir(b, c, U_raw, gamma_raw, delays):
    """Build (2,2,IR_LEN) float32 FDN impulse response."""
    delays_f = delays.astype(np.float32)
    F_ = IR_LEN // 2 + 1
    gamma_max = np.float32(10.0) ** (np.float32(-60.0 / SR / T60 / 20.0)
                                     * delays_f)               # (D,)
    gamma = (1.0 / (1.0 + np.exp(-gamma_raw.astype(np.float32)))) * gamma_max  # (S,D)
    S = gamma.shape[0]
    pos = np.arange(F_, dtype=np.float32) * np.float32((S - 1) / (F_ - 1))
    lo = np.clip(np.floor(pos).astype(np.int32), 0, S - 2)
    frac = (pos - lo.astype(np.float32))[:, None]
    g = gamma[lo] * (1.0 - frac) + gamma[lo + 1] * frac         # (F,D) fp32

    tri = np.triu(U_raw.astype(np.float64), 1)
    U = _expm(tri - tri.T).astype(np.float32)                   # (D,D)

    A = U[None, :, :] * g[:, None, :]                           # (F,D,D)
    freqs = (np.arange(F_, dtype=np.float32) / np.float32(IR_LEN)
             * np.float32(2.0 * np.pi))
    phase = freqs[:, None] * delays_f[None, :]                  # fp32 like ref
    invD = np.exp(1j * phase.astype(np.float32)).astype(np.complex64)  # (F,D)
    eye = np.eye(D, dtype=np.complex64)
    M = invD[:, :, None] * eye[None] - A.astype(np.complex64)   # (F,D,D)
    b_c = np.broadcast_to(b.astype(np.complex64), (F_, D, 2))
    X = np.linalg.solve(M, b_c)                                 # (F,D,2)
    H = np.einsum('oi,fik->fok', c.astype(np.complex64), X)     # (F,2,2)
    h = np.fft.irfft(H.transpose(1, 2, 0), n=IR_LEN, axis=-1)   # (2,2,L)
    return h.astype(np.float32)





FP = mybir.dt.float32
HP = mybir.dt.float16
XP_NP = np.float16

N = 131072
N1, N2 = 256, 512
K_IR = 44100
L_HOP = N - K_IR + 1            # 86973
T_SIG = 441000
NBLK = 6
PAD_PRE = K_IR - 1              # 44099
PAD_LEN = (NBLK - 1) * L_HOP + N  # 565937
NB = 4                          # batches per core
NIT = NB * NBLK                 # 24

# ---- per-iteration semaphore increment schedules -------------------------
PE_PER = 40   # A0=1, T0=8, C0=4, A1=1, T1=8, C1=4, Ap=4, Tp=8, Cp=2
DVE_PER = 16  # tw0=2, fdl0=4, tw1=2, fdl1=4, twp=4
ACT_PER = 28  # cT0=8, cT1=8, cS=8, cy=4


def pe_m(it):
    b = PE_PER * it
    return dict(A0=b + 1, T0=[b + 1 + k for k in range(1, 9)],
                C0=[b + 9 + k for k in range(1, 5)], A1=b + 14,
                T1=[b + 14 + k for k in range(1, 9)],
                C1=[b + 22 + k for k in range(1, 5)],
                Ap=[b + 26 + k for k in range(1, 5)],
                Tp=[b + 30 + k for k in range(1, 9)],
                Cp=[b + 38 + k for k in range(1, 3)])


def dve_m(it):
    b = DVE_PER * it
    return dict(tw0=[b + k for k in range(1, 3)],
                fdl0=[b + 2 + k for k in range(1, 5)],
                tw1=[b + 6 + k for k in range(1, 3)],
                fdl1=[b + 8 + k for k in range(1, 5)],
                twp=[b + 12 + k for k in range(1, 5)])


def act_m(it):
    b = ACT_PER * it
    return dict(cT0=[b + k for k in range(1, 9)],
                cT1=[b + 8 + k for k in range(1, 9)],
                cS=[b + 16 + k for k in range(1, 9)],
                cy=[b + 24 + k for k in range(1, 5)])


def out_ranges(j):
    """DMA row ranges for valid region of block j.
    Returns list of (n1c, row_lo, row_hi, col_lo, col_hi, dest_off) where
    rows are local to chunk n1c, dest_off is offset into y[b,ch] flat."""
    valid = min(L_HOP, T_SIG - j * L_HOP)
    p0, p1 = PAD_PRE, PAD_PRE + valid
    res = []
    for n1c in range(2):
        base = 128 * n1c
        segs = []
        for R in range(base, base + 128):
            r0, r1 = 512 * R, 512 * R + 512
            s, e = max(r0, p0), min(r1, p1)
            if s >= e:
                continue
            segs.append((R, s - r0, e - r0, s))
        # merge full rows into spans
        i = 0
        while i < len(segs):
            R, c0, c1, dst = segs[i]
            if c0 == 0 and c1 == 512:
                k = i
                while (k + 1 < len(segs) and segs[k + 1][1] == 0
                       and segs[k + 1][2] == 512):
                    k += 1
                res.append((n1c, segs[i][0] - base, segs[k][0] - base + 1,
                            0, 512, dst - p0 + j * L_HOP))
                i = k + 1
            else:
                res.append((n1c, R - base, R - base + 1, c0, c1,
                            dst - p0 + j * L_HOP))
                i += 1
    return res


def n_out_dmas(j):
    return 2 * len(out_ranges(j))   # x2 planes


def build_nc():
    nc = bass.Bass()
    xp_in = nc.declare_dram_parameter("xp", [NB, 2, PAD_LEN], HP, isOutput=False)
    w256_in = nc.declare_dram_parameter("w256", [128, 12 * 128], HP, isOutput=False)
    w512_in = nc.declare_dram_parameter("w512", [128, 48 * 128], HP, isOutput=False)
    tw_in = nc.declare_dram_parameter("tw", [128, 4 * N2], HP, isOutput=False)
    twi_in = nc.declare_dram_parameter("twi", [128, 8 * 256], HP, isOutput=False)
    g_in = nc.declare_dram_parameter("g", [128, 16 * 256], HP, isOutput=False)
    y_out = nc.declare_dram_parameter("y", [NB, 2, T_SIG], FP, isOutput=True)

    # dma counter schedule: prologue tables = 6 DMAs (incl identity? no)
    NTAB = 5
    dma_v = [16 * NTAB]  # value after prologue
    ld_after_ch0 = {}
    ld_after = {}
    out_after = {}
    v = 16 * NTAB
    for it in range(NIT):
        v += 32; ld_after_ch0[it] = v
        v += 32; ld_after[it] = v
        j = it % NBLK
        v += 16 * n_out_dmas(j); out_after[it] = v

    from contextlib import ExitStack
    es = ExitStack()
    with es:
        w256 = es.enter_context(nc.sbuf_tensor([128, 12 * 128], HP))
        w512 = es.enter_context(nc.sbuf_tensor([128, 48 * 128], HP))
        tw = es.enter_context(nc.sbuf_tensor([128, 4 * N2], HP))
        twi = es.enter_context(nc.sbuf_tensor([128, 8 * 256], HP))
        gtab = es.enter_context(nc.sbuf_tensor([128, 16 * 256], HP))
        ident = es.enter_context(nc.sbuf_tensor([128, 128], FP))
        m1 = es.enter_context(nc.sbuf_tensor([128, 2 * 2048], HP))
        t2 = es.enter_context(nc.sbuf_tensor([128, 4 * N2], FP))
        t2t = es.enter_context(nc.sbuf_tensor([128, 8 * 256], HP))
        ymat = es.enter_context(nc.sbuf_tensor([128, 8 * 256], HP))
        s2 = es.enter_context(nc.sbuf_tensor([128, 8 * 256], FP))
        s2t = es.enter_context(nc.sbuf_tensor([128, 4 * N2], HP))
        ysb = es.enter_context(nc.sbuf_tensor([128, 2 * 2048], FP))
        dvetmp = es.enter_context(nc.sbuf_tensor([128, 2 * N2], FP))
        pbanks = [es.enter_context(nc.psum_tensor(f"pb{i}", [128, N2], FP)) for i in range(8)]
        (pb0, pb1, pb2, pb3, pb4, pb5, pb6, pb7) = pbanks
        s_dma = es.enter_context(nc.semaphore("s_dma"))
        s_pe = es.enter_context(nc.semaphore("s_pe"))
        s_dve = es.enter_context(nc.semaphore("s_dve"))
        s_act = es.enter_context(nc.semaphore("s_act"))
        s_gp = es.enter_context(nc.semaphore("s_gp"))
        block = es.enter_context(nc.Block())

        def R(ap):
            return ap.bitcast(mybir.dt.float32r)

        psA = [pb0, pb1, pb2, pb3]   # re0,im0,re1,im1 (fwd) | n2c (inv, re|im packed)
        psC = [pb4, pb5, pb6, pb7]   # k2c (fwd, re|im packed) | n1c*2+plane (inv)
        psT = [pb0, pb1]             # transpose slots (alternate)

        # table slice helpers
        def w256c(n1c, k1c, plane):  # plane 0=re,1=im,2=imn
            o = (plane * 4 + n1c * 2 + k1c) * 128
            return w256[:, o:o + 128]

        def w512c(n2c, k2c, plane):
            o = (plane * 16 + n2c * 4 + k2c) * 128
            return w512[:, o:o + 128]

        def twc(k1c, plane):
            return tw[:, (plane * 2 + k1c) * N2:(plane * 2 + k1c + 1) * N2]

        def twic(n2c, plane):
            return twi[:, (plane * 4 + n2c) * 256:(plane * 4 + n2c + 1) * 256]

        def gc(i, k2c, plane):
            o = ((i * 2 + plane) * 4 + k2c) * 256
            return gtab[:, o:o + 256]

        def t2c(k1c, plane):
            return t2[:, (plane * 2 + k1c) * N2:(plane * 2 + k1c + 1) * N2]

        def t2tc(n2c, plane):
            return t2t[:, (plane * 4 + n2c) * 256:(plane * 4 + n2c + 1) * 256]

        def ymc(k2c, plane):
            return ymat[:, (plane * 4 + k2c) * 256:(plane * 4 + k2c + 1) * 256]

        def s2c(n2c, plane):
            return s2[:, (plane * 4 + n2c) * 256:(plane * 4 + n2c + 1) * 256]

        def s2tc(k1c, plane):
            return s2t[:, (plane * 2 + k1c) * N2:(plane * 2 + k1c + 1) * N2]

        @block.gpsimd
        def _(gpsimd):
            make_identity(nc, ident[:, :])
            gpsimd.sem_inc(s_gp, 1)

        def emit_outs(sync, it):
            b, j = divmod(it, NBLK)
            buf = it % 2
            sync.wait_ge(s_act, act_m(it)["cy"][3])
            for plane in range(2):          # 0=re->ch0, 1=im->ch1
                for (n1c, rlo, rhi, clo, chi, doff) in out_ranges(j):
                    o_ = buf * 2048 + (n1c * 2 + plane) * N2
                    src = ysb[rlo:rhi, o_ + clo:o_ + chi]
                    cnt = (rhi - rlo) * (chi - clo)
                    dst = y_out[b, plane, doff:doff + cnt]
                    dst = dst.rearrange("(p f) -> p f", f=chi - clo)
                    sync.dma_start(dst, src).then_inc(s_dma, 16)

        @block.sync
        def _(sync):
            sync.dma_start(w256[:, :], w256_in[:, :]).then_inc(s_dma, 16)
            sync.dma_start(w512[:, :], w512_in[:, :]).then_inc(s_dma, 16)
            sync.dma_start(tw[:, :], tw_in[:, :]).then_inc(s_dma, 16)
            sync.dma_start(twi[:, :], twi_in[:, :]).then_inc(s_dma, 16)
            sync.dma_start(gtab[:, :], g_in[:, :]).then_inc(s_dma, 16)
            for it in range(NIT):
                b, j = divmod(it, NBLK)
                buf = it % 2
                if it >= 2:
                    sync.wait_ge(s_pe, pe_m(it - 2)["A1"])
                for ch in range(2):
                    src = xp_in[b, ch, j * L_HOP: j * L_HOP + N]
                    src = src.rearrange("(p f) -> p f", f=N2)
                    for n1c in range(2):
                        o_ = buf * 2048 + (ch * 2 + n1c) * N2
                        sync.dma_start(
                            m1[:, o_:o_ + N2],
                            src[128 * n1c:128 * (n1c + 1), :],
                        ).then_inc(s_dma, 16)
                # outputs of previous iteration
                if it >= 1:
                    emit_outs(sync, it - 1)
            emit_outs(sync, NIT - 1)

        @block.tensor
        def _(tensor):
            tensor.wait_ge(s_gp, 1)
            tensor.wait_ge(s_dma, 16 * 5)
            for it in range(NIT):
                pm, dm, am = pe_m(it), dve_m(it), act_m(it)
                buf = it % 2

                def m1c(ch, n1c):
                    o_ = buf * 2048 + (ch * 2 + n1c) * N2
                    return m1[:, o_:o_ + N2]

                # ---------------- forward FFT per channel ----------------
                for ch in range(2):
                    # stage A: T1[k1c] (re->psA[2*k1c?] layout re0,im0,re1,im1)
                    if ch == 0:
                        tensor.wait_ge(s_dma, ld_after_ch0[it])
                        if it >= 1:
                            tensor.wait_ge(s_dve, dve_m(it - 1)["twp"][3])
                            tensor.wait_ge(s_act, act_m(it - 1)["cS"][7])
                    else:
                        tensor.wait_ge(s_dma, ld_after[it])
                        tensor.wait_ge(s_dve, dm["tw0"][1])
                        tensor.wait_ge(s_act, am["cT0"][7])
                    last = None
                    for k1c in range(2):
                        pre, pim = psA[2 * k1c], psA[2 * k1c + 1]
                        for n1c in range(2):
                            nc.tensor.matmul(pre[:, :], w256c(n1c, k1c, 0),
                                             m1c(ch, n1c), start=(n1c == 0),
                                             stop=(n1c == 1))
                        for n1c in range(2):
                            last = nc.tensor.matmul(
                                pim[:, :], w256c(n1c, k1c, 1), m1c(ch, n1c),
                                start=(n1c == 0), stop=(n1c == 1))
                    last.then_inc(s_pe, 1)      # A done
                    # transposes: pairs (n2c,k1c-combined): 8 pairs of (re,im)
                    twk = dm["tw0"] if ch == 0 else dm["tw1"]
                    cTk = am["cT0"] if ch == 0 else am["cT1"]
                    p = 0
                    for n2c in range(4):
                        for k1c in range(2):
                            tensor.wait_ge(s_dve, twk[k1c])
                            if p >= 2:
                                tensor.wait_ge(s_act, cTk[p - 2])
                            slot = psT[p % 2]
                            nc.tensor.transpose(
                                slot[:, 0:128],
                                t2c(k1c, 0)[:, n2c * 128:(n2c + 1) * 128],
                                ident[:, :])
                            nc.tensor.transpose(
                                slot[:, 128:256],
                                t2c(k1c, 1)[:, n2c * 128:(n2c + 1) * 128],
                                ident[:, :]).then_inc(s_pe, 1)
                            p += 1
                    # stage C: X[k2c] re|im packed into psC[k2c]
                    tensor.wait_ge(s_act, cTk[7])
                    if ch == 0 and it >= 1:
                        tensor.wait_ge(s_act, act_m(it - 1)["cy"][3])
                    if ch == 1:
                        tensor.wait_ge(s_dve, dm["fdl0"][3])
                    for k2c in range(4):
                        pre = psC[k2c][:, 0:256]
                        pim = psC[k2c][:, 256:512]
                        seq = []
                        for n2c in range(4):
                            seq.append((pre, w512c(n2c, k2c, 0), t2tc(n2c, 0)))
                        for n2c in range(4):
                            seq.append((pre, w512c(n2c, k2c, 2), t2tc(n2c, 1)))
                        for i_, (dst, w_, r_) in enumerate(seq):
                            nc.tensor.matmul(dst, w_, r_, start=(i_ == 0),
                                             stop=(i_ == 7))
                        seq = []
                        for n2c in range(4):
                            seq.append((pim, w512c(n2c, k2c, 0), t2tc(n2c, 1)))
                        for n2c in range(4):
                            seq.append((pim, w512c(n2c, k2c, 1), t2tc(n2c, 0)))
                        for i_, (dst, w_, r_) in enumerate(seq):
                            mm = nc.tensor.matmul(dst, w_, r_, start=(i_ == 0),
                                                  stop=(i_ == 7))
                        mm.then_inc(s_pe, 1)
                # ---------------- inverse ----------------
                # stage A': S[n2c] = V512 @ Ymat ; V512: re=w512re, im=w512imn, imn=w512im
                tensor.wait_ge(s_dve, dm["fdl1"][3])
                tensor.wait_ge(s_dve, dm["tw1"][1])
                tensor.wait_ge(s_act, am["cT1"][7])
                for n2c in range(4):
                    pre = psA[n2c][:, 0:256]
                    pim = psA[n2c][:, 256:512]
                    seq = []
                    for k2c in range(4):
                        seq.append((pre, w512c(k2c, n2c, 0), ymc(k2c, 0)))
                    for k2c in range(4):
                        seq.append((pre, w512c(k2c, n2c, 1), ymc(k2c, 1)))
                    for i_, (dst, w_, r_) in enumerate(seq):
                        nc.tensor.matmul(dst, w_, r_, start=(i_ == 0),
                                         stop=(i_ == 7))
                    seq = []
                    for k2c in range(4):
                        seq.append((pim, w512c(k2c, n2c, 0), ymc(k2c, 1)))
                    for k2c in range(4):
                        seq.append((pim, w512c(k2c, n2c, 2), ymc(k2c, 0)))
                    for i_, (dst, w_, r_) in enumerate(seq):
                        mm = nc.tensor.matmul(dst, w_, r_, start=(i_ == 0),
                                              stop=(i_ == 7))
                    mm.then_inc(s_pe, 1)
                # transposes of S2 -> psT -> (ACT) s2t
                p = 0
                for n2c in range(4):
                    for k1c in range(2):
                        tensor.wait_ge(s_dve, dm["twp"][max(n2c, p % 2)])
                        if p >= 2:
                            tensor.wait_ge(s_act, am["cS"][p - 2])
                        slot = psT[p % 2]
                        nc.tensor.transpose(
                            slot[:, 0:128],
                            s2c(n2c, 0)[:, k1c * 128:(k1c + 1) * 128],
                            ident[:, :])
                        nc.tensor.transpose(
                            slot[:, 128:256],
                            s2c(n2c, 1)[:, k1c * 128:(k1c + 1) * 128],
                            ident[:, :]).then_inc(s_pe, 1)
                        p += 1
                # stage C': y[n1c] = V256 @ S2t ; V256: re=w256re, im=w256imn->plane2? V=conj
                tensor.wait_ge(s_act, am["cS"][7])
                for n1c in range(2):
                    pre = psC[2 * n1c]      # full 512 wide (re plane)
                    pim = psC[2 * n1c + 1]
                    seq = []
                    for k1c in range(2):
                        seq.append((pre, w256c(k1c, n1c, 0), s2tc(k1c, 0)))
                    for k1c in range(2):
                        seq.append((pre, w256c(k1c, n1c, 1), s2tc(k1c, 1)))
                    for i_, (dst, w_, r_) in enumerate(seq):
                        nc.tensor.matmul(dst[:, :], w_, r_, start=(i_ == 0),
                                         stop=(i_ == 3))
                    seq = []
                    for k1c in range(2):
                        seq.append((pim, w256c(k1c, n1c, 0), s2tc(k1c, 1)))
                    for k1c in range(2):
                        seq.append((pim, w256c(k1c, n1c, 2), s2tc(k1c, 0)))
                    for i_, (dst, w_, r_) in enumerate(seq):
                        mm = nc.tensor.matmul(dst[:, :], w_, r_, start=(i_ == 0),
                                              stop=(i_ == 3))
                    mm.then_inc(s_pe, 1)

        @block.vector
        def _(vector):
            for it in range(NIT):
                pm, dm, am = pe_m(it), dve_m(it), act_m(it)

                def cmul_psum(dst_re, dst_im, src_re, src_im, twr, twi_,
                              tmp1, tmp2):
                    nc.vector.tensor_mul(dst_re, src_re, twr)
                    nc.vector.tensor_mul(tmp1, src_im, twi_)
                    nc.vector.tensor_sub(dst_re, dst_re, tmp1)
                    nc.vector.tensor_mul(dst_im, src_re, twi_)
                    nc.vector.tensor_mul(tmp2, src_im, twr)
                    return nc.vector.tensor_add(dst_im, dst_im, tmp2)

                for ch in range(2):
                    Ad = pm["A0"] if ch == 0 else pm["A1"]
                    twk = dm["tw0"] if ch == 0 else dm["tw1"]
                    vector.wait_ge(s_pe, Ad)
                    for k1c in range(2):
                        cmul_psum(t2c(k1c, 0), t2c(k1c, 1),
                                  psA[2 * k1c][:, :], psA[2 * k1c + 1][:, :],
                                  twc(k1c, 0), twc(k1c, 1),
                                  dvetmp[:, 0:N2], dvetmp[:, N2:2 * N2]
                                  ).then_inc(s_dve, 1)
                    # FDL
                    Ck = pm["C0"] if ch == 0 else pm["C1"]
                    for k2c in range(4):
                        vector.wait_ge(s_pe, Ck[k2c])
                        xre = psC[k2c][:, 0:256]
                        xim = psC[k2c][:, 256:512]
                        t1 = dvetmp[:, 0:256]
                        if ch == 0:
                            nc.vector.tensor_mul(ymc(k2c, 0), xre, gc(0, k2c, 0))
                            nc.vector.tensor_mul(t1, xim, gc(0, k2c, 1))
                            nc.vector.tensor_sub(ymc(k2c, 0), ymc(k2c, 0), t1)
                            nc.vector.tensor_mul(ymc(k2c, 1), xre, gc(0, k2c, 1))
                            nc.vector.tensor_mul(t1, xim, gc(0, k2c, 0))
                            nc.vector.tensor_add(ymc(k2c, 1), ymc(k2c, 1), t1
                                                 ).then_inc(s_dve, 1)
                        else:
                            nc.vector.tensor_mul(t1, xre, gc(1, k2c, 0))
                            nc.vector.tensor_add(ymc(k2c, 0), ymc(k2c, 0), t1)
                            nc.vector.tensor_mul(t1, xim, gc(1, k2c, 1))
                            nc.vector.tensor_sub(ymc(k2c, 0), ymc(k2c, 0), t1)
                            nc.vector.tensor_mul(t1, xre, gc(1, k2c, 1))
                            nc.vector.tensor_add(ymc(k2c, 1), ymc(k2c, 1), t1)
                            nc.vector.tensor_mul(t1, xim, gc(1, k2c, 0))
                            nc.vector.tensor_add(ymc(k2c, 1), ymc(k2c, 1), t1
                                                 ).then_inc(s_dve, 1)
                # inverse twiddle: S2[n2c] = S[n2c] * TWI[n2c]
                for n2c in range(4):
                    vector.wait_ge(s_pe, pm["Ap"][n2c])
                    cmul_psum(s2c(n2c, 0), s2c(n2c, 1),
                              psA[n2c][:, 0:256], psA[n2c][:, 256:512],
                              twic(n2c, 0), twic(n2c, 1),
                              dvetmp[:, 0:256], dvetmp[:, 256:512]
                              ).then_inc(s_dve, 1)

        @block.scalar
        def _(scalar):
            for it in range(NIT):
                pm, am = pe_m(it), act_m(it)
                buf = it % 2
                for ch in range(2):
                    Tk = pm["T0"] if ch == 0 else pm["T1"]
                    p = 0
                    for n2c in range(4):
                        for k1c in range(2):
                            scalar.wait_ge(s_pe, Tk[p])
                            slot = psT[p % 2]
                            nc.scalar.copy(
                                t2tc(n2c, 0)[:, k1c * 128:(k1c + 1) * 128],
                                slot[:, 0:128])
                            nc.scalar.copy(
                                t2tc(n2c, 1)[:, k1c * 128:(k1c + 1) * 128],
                                slot[:, 128:256]).then_inc(s_act, 1)
                            p += 1
                # inverse transpose copies
                p = 0
                for n2c in range(4):
                    for k1c in range(2):
                        scalar.wait_ge(s_pe, pm["Tp"][p])
                        slot = psT[p % 2]
                        nc.scalar.copy(
                            s2tc(k1c, 0)[:, n2c * 128:(n2c + 1) * 128],
                            slot[:, 0:128])
                        nc.scalar.copy(
                            s2tc(k1c, 1)[:, n2c * 128:(n2c + 1) * 128],
                            slot[:, 128:256]).then_inc(s_act, 1)
                        p += 1
                # y copies psC -> ysb
                if it >= 2:
                    scalar.wait_ge(s_dma, out_after[it - 2])
                for n1c in range(2):
                    scalar.wait_ge(s_pe, pm["Cp"][n1c])
                    ob = buf * 2048
                    nc.scalar.copy(ysb[:, ob + (n1c * 2 + 0) * N2:ob + (n1c * 2 + 1) * N2],
                                   psC[2 * n1c][:, :])
                    nc.scalar.copy(ysb[:, ob + (n1c * 2 + 1) * N2:ob + (n1c * 2 + 2) * N2],
                                   psC[2 * n1c + 1][:, :]).then_inc(s_act, 1)
                    scalar.sem_inc(s_act, 1)   # cy has 4 slots: 2 per n1c
    return nc


# ---------------------------- host side ----------------------------------

def make_device_tables(h):
    """h: (2,2,K_IR) float32 -> dict of DRAM table arrays (fp32)."""
    def dftm(n, sign):
        k = np.arange(n)
        return np.exp(sign * 2j * np.pi * np.outer(k, k) / n)
    W256 = dftm(N1, -1)
    W512 = dftm(N2, -1)
    k1 = np.arange(N1)
    n2 = np.arange(N2)
    TW = np.exp(-2j * np.pi * np.outer(k1, n2) / N)
    TWI = np.exp(+2j * np.pi * np.outer(n2, k1) / N)

    def chunks(M, pr, pc, planes):   # planes: list of 2d arrays [R,C]
        # returns [128, len(planes)*pr*pc*...] col-concatenated in
        # (plane, rowchunk, colchunk) order with 128-col chunks
        cols = []
        for P in planes:
            for a in range(pr):
                for b_ in range(pc):
                    cols.append(P[128 * a:128 * (a + 1),
                                  128 * b_:128 * (b_ + 1)].astype(np.float32))
        return np.ascontiguousarray(np.concatenate(cols, axis=1))

    w256 = chunks(None, 2, 2, [W256.real, W256.imag, -W256.imag])
    w512 = chunks(None, 4, 4, [W512.real, W512.imag, -W512.imag])

    twp = np.concatenate([TW.real[0:128], TW.real[128:256],
                          TW.imag[0:128], TW.imag[128:256]], axis=1)
    twip = np.concatenate([TWI.real[128 * a:128 * (a + 1)] for a in range(4)]
                          + [TWI.imag[128 * a:128 * (a + 1)] for a in range(4)],
                          axis=1)
    hp = np.zeros((2, 2, N), np.float64)
    hp[:, :, :K_IR] = h
    gcols = []
    for i in range(2):
        G = (np.fft.fft(hp[0, i]) + 1j * np.fft.fft(hp[1, i])) / N
        Gm = G.reshape(N2, N1)          # [k2, k1]
        for plane in range(2):
            P = Gm.real if plane == 0 else Gm.imag
            for k2c in range(4):
                gcols.append(P[128 * k2c:128 * (k2c + 1), :].astype(np.float32))
    g = np.ascontiguousarray(np.concatenate(gcols, axis=1))
    return dict(w256=w256.astype(np.float16), w512=w512.astype(np.float16),
                tw=twp.astype(np.float16), twi=twip.astype(np.float16),
                g=g.astype(np.float16))


_NC_CACHE = None


def make_in_maps(x, h):
    """Per-core input maps for the SPMD launch."""
    B = x.shape[0]
    assert B == 8 * NB
    xp = np.zeros((B, 2, PAD_LEN), np.float16)
    xp[:, :, PAD_PRE:PAD_PRE + T_SIG] = x.astype(np.float16)
    tabs = make_device_tables(h)
    in_maps = []
    for c in range(8):
        m = {"xp": xp[NB * c:NB * (c + 1)]}
        m.update(tabs)
        in_maps.append(m)
    return in_maps


def conv_device(x, h):
    """x: (B,2,T) fp32, h: (2,2,K_IR) fp32 -> y: (B,2,T) fp32 via 8 cores."""
    global _NC_CACHE
    from concourse.bass_utils import run_bass_kernel_spmd
    if _NC_CACHE is None:
        _NC_CACHE = build_nc()
    nc = _NC_CACHE
    in_maps = make_in_maps(x, h)
    res = run_bass_kernel_spmd(nc, in_maps, list(range(8)))
    y = np.concatenate([res.results[c]["y"] for c in range(8)], axis=0)
    return y




def kernel(**inputs):
    """Full FDN: build IR on host, FFT-convolve on 8 NeuronCores."""
    x = np.asarray(inputs["x"], np.float32)
    h = fdn_ir(np.asarray(inputs["b"]), np.asarray(inputs["c"]),
               np.asarray(inputs["U_raw"]), np.asarray(inputs["gamma_raw"]),
               np.asarray(inputs["delays"]))
    y = conv_device(x, h)
    return y.astype(np.float32)



# revision 10
# speedup vs baseline: 3.2081x; 1.6181x over previous
"""Self-contained Trainium2 kernel for nn_FDN_37211596653125 (v2).

kernel(**inputs) -> y (32,2,441000) float32.
Host: FDN impulse response (tiny 6x6 solves). Device (8 NeuronCores, raw
Bass): overlap-save FFT convolution, N=131072=256x512 Cooley-Tukey via PE
matmuls in fp16 (fp32 PSUM accumulate), batch-PAIRED complex FFTs
(z = x[2p] + i*x[2p+1] per channel; by linearity one complex FFT serves
two batches), twiddle/spectral products on VectorE in fp16 2x mode after
Act-engine PSUM evacuation, fp16 PE transposes into bitcast PSUM views.
"""
import sys
sys.path.insert(0, "/opt/trn_rl_repo")
import numpy as np
import concourse.bass as bass
import concourse.mybir as mybir
from concourse.masks import make_identity


SR = 44100
IR_LEN = 44100
T60 = 0.75
D = 6


def _expm(A):
    """Pade-13 scaling-and-squaring matrix exponential (float64)."""
    A = A.astype(np.float64)
    b = [64764752532480000.0, 32382376266240000.0, 7771770303897600.0,
         1187353796428800.0, 129060195264000.0, 10559470521600.0,
         670442572800.0, 33522128640.0, 1323241920.0, 40840800.0,
         960960.0, 16380.0, 182.0, 1.0]
    n = A.shape[0]
    nrm = np.linalg.norm(A, 1)
    theta13 = 5.371920351148152
    s = max(0, int(np.ceil(np.log2(max(nrm / theta13, 1e-300)))))
    if nrm <= theta13:
        s = 0
    A = A / (2.0 ** s)
    I = np.eye(n)
    A2 = A @ A
    A4 = A2 @ A2
    A6 = A2 @ A4
    U = A @ (A6 @ (b[13] * A6 + b[11] * A4 + b[9] * A2)
             + b[7] * A6 + b[5] * A4 + b[3] * A2 + b[1] * I)
    V = (A6 @ (b[12] * A6 + b[10] * A4 + b[8] * A2)
         + b[6] * A6 + b[4] * A4 + b[2] * A2 + b[0] * I)
    R = np.linalg.solve(V - U, V + U)
    for _ in range(s):
        R = R @ R
    return R


def fdn_ir(b, c, U_raw, gamma_raw, delays):
    """Build (2,2,IR_LEN) float32 FDN impulse response."""
    delays_f = delays.astype(np.float32)
    F_ = IR_LEN // 2 + 1
    gamma_max = np.float32(10.0) ** (np.float32(-60.0 / SR / T60 / 20.0)
                                     * delays_f)               # (D,)
    gamma = (1.0 / (1.0 + np.exp(-gamma_raw.astype(np.float32)))) * gamma_max  # (S,D)
    S = gamma.shape[0]
    pos = np.arange(F_, dtype=np.float32) * np.float32((S - 1) / (F_ - 1))
    lo = np.clip(np.floor(pos).astype(np.int32), 0, S - 2)
    frac = (pos - lo.astype(np.float32))[:, None]
    g = gamma[lo] * (1.0 - frac) + gamma[lo + 1] * frac         # (F,D) fp32

    tri = np.triu(U_raw.astype(np.float64), 1)
    U = _expm(tri - tri.T).astype(np.float32)                   # (D,D)

    A = U[None, :, :] * g[:, None, :]                           # (F,D,D)
    freqs = (np.arange(F_, dtype=np.float32) / np.float32(IR_LEN)
             * np.float32(2.0 * np.pi))
    phase = freqs[:, None] * delays_f[None, :]                  # fp32 like ref
    invD = np.exp(1j * phase.astype(np.float32)).astype(np.complex64)  # (F,D)
    eye = np.eye(D, dtype=np.complex64)
    M = invD[:, :, None] * eye[None] - A.astype(np.complex64)   # (F,D,D)
    b_c = np.broadcast_to(b.astype(np.complex64), (F_, D, 2))
    X = np.linalg.solve(M, b_c)                                 # (F,D,2)
    H = np.einsum('oi,fik->fok', c.astype(np.complex64), X)     # (F,2,2)
    h = np.fft.irfft(H.transpose(1, 2, 0), n=IR_LEN, axis=-1)   # (2,2,L)
    return h.astype(np.float32)


FP = mybir.dt.float32
HP = mybir.dt.float16
XP_NP = np.float16

N = 131072
N1, N2 = 256, 512
K_IR = 44100
L_HOP = N - K_IR + 1            # 86973
T_SIG = 441000
NBLK = 6
PAD_PRE = K_IR - 1              # 44099
PAD_LEN = (NBLK - 1) * L_HOP + N  # 565937
NB = 4                          # batches per core
NPAIR = 2                       # batch pairs per core
NIT = NPAIR * NBLK              # 12
FWD_ONLY = False                # debug: skip inverse half

# ---- per-iteration semaphore increment schedules -------------------------
PE_PER = 40   # A0=2,T0=4,C0=4,A1=2,T1=4,C1=4,Ap=4,Ap2=4,Tp=4,Cp=2,Tp2=4,Cp2=2
DVE_PER = 11  # tw0=2, fdl0=1, tw1=2, fdl1=2, twp=2, twp2=2
ACT_PER = 44  # tA0=4,cT0=4,cX0=4,tA1=4,cT1=4,cX1=4,sA=4,cS=4,cy=2,sA2=4,cS2=4,cy2=2


def pe_m(it):
    b = PE_PER * it
    return dict(A0=[b + 1, b + 2],
                T0=[b + 2 + k for k in range(1, 5)],
                C0=[b + 6 + k for k in range(1, 5)],
                A1=[b + 11, b + 12],
                T1=[b + 12 + k for k in range(1, 5)],
                C1=[b + 16 + k for k in range(1, 5)],
                Ap=[b + 20 + k for k in range(1, 5)],
                Ap2=[b + 24 + k for k in range(1, 5)],
                Tp=[b + 28 + k for k in range(1, 5)],
                Cp=[b + 33, b + 34],
                Tp2=[b + 34 + k for k in range(1, 5)],
                Cp2=[b + 39, b + 40])


def dve_m(it):
    b = DVE_PER * it
    return dict(tw0=[b + 1, b + 2], fdl0=b + 3, tw1=[b + 4, b + 5],
                fdl1=[b + 6, b + 7], twp=[b + 8, b + 9],
                twp2=[b + 10, b + 11])


def act_m(it):
    b = ACT_PER * it
    return dict(tA0=[b + k for k in range(1, 5)],
                cT0=[b + 4 + k for k in range(1, 5)],
                cX0=[b + 8 + k for k in range(1, 5)],
                tA1=[b + 12 + k for k in range(1, 5)],
                cT1=[b + 16 + k for k in range(1, 5)],
                cX1=[b + 20 + k for k in range(1, 5)],
                sA=[b + 24 + k for k in range(1, 5)],
                cS=[b + 28 + k for k in range(1, 5)],
                cy=[b + 33, b + 34],
                sA2=[b + 34 + k for k in range(1, 5)],
                cS2=[b + 38 + k for k in range(1, 5)],
                cy2=[b + 43, b + 44])


def out_ranges(j):
    """DMA row ranges for valid region of block j (rows local to n1c chunk)."""
    valid = min(L_HOP, T_SIG - j * L_HOP)
    p0, p1 = PAD_PRE, PAD_PRE + valid
    res = []
    for n1c in range(2):
        base = 128 * n1c
        segs = []
        for R in range(base, base + 128):
            r0, r1 = 512 * R, 512 * R + 512
            s, e = max(r0, p0), min(r1, p1)
            if s >= e:
                continue
            segs.append((R, s - r0, e - r0, s))
        i = 0
        while i < len(segs):
            R, c0, c1, dst = segs[i]
            if c0 == 0 and c1 == 512:
                k = i
                while (k + 1 < len(segs) and segs[k + 1][1] == 0
                       and segs[k + 1][2] == 512):
                    k += 1
                res.append((n1c, segs[i][0] - base, segs[k][0] - base + 1,
                            0, 512, dst - p0 + j * L_HOP))
                i = k + 1
            else:
                res.append((n1c, R - base, R - base + 1, c0, c1,
                            dst - p0 + j * L_HOP))
                i += 1
    return res


def n_out_dmas(j):
    return 4 * len(out_ranges(j))   # 2 outputs x 2 planes (batches)


def build_nc():
    nc = bass.Bass(detect_race_conditions=False)
    xp_in = nc.declare_dram_parameter("xp", [NB, 2, PAD_LEN], HP, isOutput=False)
    w256_in = nc.declare_dram_parameter("w256", [128, 12 * 128], HP, isOutput=False)
    w512_in = nc.declare_dram_parameter("w512", [128, 48 * 128], HP, isOutput=False)
    tw_in = nc.declare_dram_parameter("tw", [128, 2048], HP, isOutput=False)
    twi_in = nc.declare_dram_parameter("twi", [128, 2048], HP, isOutput=False)
    g_in = nc.declare_dram_parameter("g", [128, 8192], HP, isOutput=False)
    y_out = nc.declare_dram_parameter("y", [NB, 2, T_SIG], FP, isOutput=True)

    NTAB = 5
    ld_ch0 = {}
    ld_all = {}
    out_after = {}
    v = 16 * NTAB
    for it in range(NIT):
        v += 64; ld_ch0[it] = v          # 4 tiles ch0
        v += 64; ld_all[it] = v          # 4 tiles ch1
        j = it % NBLK
        v += 16 * n_out_dmas(j); out_after[it] = v

    from contextlib import ExitStack
    es = ExitStack()
    with es:
        w256 = es.enter_context(nc.sbuf_tensor([128, 12 * 128], HP))
        w512 = es.enter_context(nc.sbuf_tensor([128, 48 * 128], HP))
        tw = es.enter_context(nc.sbuf_tensor([128, 2048], HP))
        twi = es.enter_context(nc.sbuf_tensor([128, 2048], HP))
        gtab = es.enter_context(nc.sbuf_tensor([128, 8192], HP))
        ident = es.enter_context(nc.sbuf_tensor([128, 128], HP))
        m1 = es.enter_context(nc.sbuf_tensor([128, 2 * 8 * 512], HP))
        tA = es.enter_context(nc.sbuf_tensor([128, 2048], HP))
        t2 = es.enter_context(nc.sbuf_tensor([128, 2048], HP))
        t2t = es.enter_context(nc.sbuf_tensor([128, 2048], HP))
        xsp = es.enter_context(nc.sbuf_tensor([128, 2 * 2048], HP))
        ymat = es.enter_context(nc.sbuf_tensor([128, 2 * 2048], HP))
        sr = es.enter_context(nc.sbuf_tensor([128, 2 * 2048], HP))
        s2 = es.enter_context(nc.sbuf_tensor([128, 2 * 2048], HP))
        s2t = es.enter_context(nc.sbuf_tensor([128, 2 * 2048], HP))
        ysb = es.enter_context(nc.sbuf_tensor([128, 2 * 4096], FP))
        dtmp = es.enter_context(nc.sbuf_tensor([128, 2048], HP))
        pb = [es.enter_context(nc.psum_tensor(f"pb{i}", [128, N2], FP))
              for i in range(8)]
        s_dma = es.enter_context(nc.semaphore("s_dma"))
        s_pe = es.enter_context(nc.semaphore("s_pe"))
        s_dve = es.enter_context(nc.semaphore("s_dve"))
        s_act = es.enter_context(nc.semaphore("s_act"))
        s_gp = es.enter_context(nc.semaphore("s_gp"))
        block = es.enter_context(nc.Block())

        # ---------- layout helpers ----------
        def w256c(a, bq, plane):        # [128,128] chunk (rowchunk a, colchunk bq)
            return w256[:, (plane * 4 + a * 2 + bq) * 128:
                        (plane * 4 + a * 2 + bq) * 128 + 128]

        def w512c(a, bq, plane):
            return w512[:, (plane * 16 + a * 4 + bq) * 128:
                        (plane * 16 + a * 4 + bq) * 128 + 128]

        def m1c(buf, ch, plane, n1c):
            o = buf * 4096 + ((ch * 2 + plane) * 2 + n1c) * 512
            return m1[:, o:o + 512]

        def tAc(plane):                 # [128,1024] (k1c-major)
            return tA[:, plane * 1024:plane * 1024 + 1024]

        def t2b(plane, k1c, n2c):       # [128,128] transpose input block
            o = plane * 1024 + k1c * 512 + n2c * 128
            return t2[:, o:o + 128]

        def t2full(plane):
            return t2[:, plane * 1024:plane * 1024 + 1024]

        def t2tc(n2c, plane):           # [128,256] stage-C moving operand
            o = n2c * 512 + plane * 256
            return t2t[:, o:o + 256]

        def xspc(ch):                   # [128,2048] per channel
            return xsp[:, ch * 2048:ch * 2048 + 2048]

        def ymc(o):                     # [128,2048] per output
            return ymat[:, o * 2048:o * 2048 + 2048]

        def ym_mv(o, k2c, plane):       # [128,256] A' moving operand
            return ymat[:, o * 2048 + plane * 1024 + k2c * 256:
                        o * 2048 + plane * 1024 + k2c * 256 + 256]

        def src_(o, plane):             # [128,1024] sr halves
            return sr[:, o * 2048 + plane * 1024:o * 2048 + plane * 1024 + 1024]

        def s2c(o, plane):              # [128,1024]
            return s2[:, o * 2048 + plane * 1024:o * 2048 + plane * 1024 + 1024]

        def s2b(o, plane, n2c, k1c):    # [128,128] Tp input block
            o_ = o * 2048 + plane * 1024 + n2c * 256 + k1c * 128
            return s2[:, o_:o_ + 128]

        def s2tc(o, k1c, plane):        # [128,512] C' moving operand
            o_ = o * 2048 + (k1c * 2 + plane) * 512
            return s2t[:, o_:o_ + 512]

        def gc(o, ch, plane):           # [128,1024]
            o_ = ((o * 2 + ch) * 2 + plane) * 1024
            return gtab[:, o_:o_ + 1024]

        def twc(plane):                 # [128,1024]
            return tw[:, plane * 1024:plane * 1024 + 1024]

        def twic(plane):
            return twi[:, plane * 1024:plane * 1024 + 1024]

        def ysbc(buf, o, n1c, plane):   # [128,512] fp32
            o_ = buf * 4096 + o * 2048 + (n1c * 2 + plane) * 512
            return ysb[:, o_:o_ + 512]

        def ps16(bank, chunk):          # [128,128] fp16 view of psum bank
            return pb[bank][:, chunk * 64:chunk * 64 + 64].bitcast(HP)

        @block.gpsimd
        def _(gpsimd):
            make_identity(nc, ident[:, :])
            gpsimd.sem_inc(s_gp, 1)

        # ------------------------------------------------------------------
        @block.sync
        def _(sync):
            sync.dma_start(w256[:, :], w256_in[:, :]).then_inc(s_dma, 16)
            sync.dma_start(w512[:, :], w512_in[:, :]).then_inc(s_dma, 16)
            sync.dma_start(tw[:, :], tw_in[:, :]).then_inc(s_dma, 16)
            sync.dma_start(twi[:, :], twi_in[:, :]).then_inc(s_dma, 16)
            sync.dma_start(gtab[:, :], g_in[:, :]).then_inc(s_dma, 16)

            def emit_outs(it, o):
                p_, j = divmod(it, NBLK)
                buf = it % 2
                am = act_m(it)
                sync.wait_ge(s_act, am["cy"][1] if o == 0 else am["cy2"][1])
                for plane in range(2):
                    for (n1c, rlo, rhi, clo, chi, doff) in out_ranges(j):
                        src = ysbc(buf, o, n1c, plane)[rlo:rhi, clo:chi]
                        cnt = (rhi - rlo) * (chi - clo)
                        dst = y_out[2 * p_ + plane, o, doff:doff + cnt]
                        dst = dst.rearrange("(p f) -> p f", f=chi - clo)
                        sync.dma_start(dst, src).then_inc(s_dma, 16)

            for it in range(NIT):
                p_, j = divmod(it, NBLK)
                buf = it % 2
                if it >= 2:
                    sync.wait_ge(s_pe, pe_m(it - 2)["A1"][1])
                for ch in range(2):
                    for plane in range(2):
                        src = xp_in[2 * p_ + plane, ch,
                                    j * L_HOP: j * L_HOP + N]
                        src = src.rearrange("(p f) -> p f", f=N2)
                        for n1c in range(2):
                            sync.dma_start(
                                m1c(buf, ch, plane, n1c),
                                src[128 * n1c:128 * (n1c + 1), :],
                            ).then_inc(s_dma, 16)
                if it >= 1 and not FWD_ONLY:
                    emit_outs(it - 1, 0)
                    emit_outs(it - 1, 1)
            if not FWD_ONLY:
                emit_outs(NIT - 1, 0)
                emit_outs(NIT - 1, 1)
            else:
                # debug: ensure y is written so outputs aren't pruned
                sync.wait_ge(s_act, act_m(NIT - 1)["cX1"][3])
                dst = y_out[0, 0, 0:65536].rearrange("(p f) -> p f", f=512)
                sync.dma_start(dst, m1[:, 0:1024].bitcast(FP)
                               ).then_inc(s_dma, 16)

        # ------------------------------------------------------------------
        @block.tensor
        def _(tensor):
            tensor.wait_ge(s_gp, 1)
            tensor.wait_ge(s_dma, 16 * NTAB)
            for it in range(NIT):
                pm, dm, am = pe_m(it), dve_m(it), act_m(it)
                buf = it % 2

                # =================== forward, per channel ===================
                for ch in range(2):
                    if ch == 0:
                        tensor.wait_ge(s_dma, ld_ch0[it])
                        if it >= 1:
                            tensor.wait_ge(s_act, act_m(it - 1)["cy"][1])
                    else:
                        tensor.wait_ge(s_dma, ld_all[it])
                        tensor.wait_ge(s_act, am["tA0"][3])
                    # stage A: psA banks re0=pb0 im0=pb1 re1=pb2 im1=pb3
                    for k1c in range(2):
                        pre, pim = pb[2 * k1c], pb[2 * k1c + 1]
                        seq = ([(pre, w256c(n1c, k1c, 0), m1c(buf, ch, 0, n1c))
                                for n1c in range(2)]
                               + [(pre, w256c(n1c, k1c, 2), m1c(buf, ch, 1, n1c))
                                  for n1c in range(2)])
                        for i_, (d_, w_, r_) in enumerate(seq):
                            nc.tensor.matmul(d_[:, :], w_, r_, start=(i_ == 0),
                                             stop=(i_ == 3))
                        seq = ([(pim, w256c(n1c, k1c, 1), m1c(buf, ch, 0, n1c))
                                for n1c in range(2)]
                               + [(pim, w256c(n1c, k1c, 0), m1c(buf, ch, 1, n1c))
                                  for n1c in range(2)])
                        for i_, (d_, w_, r_) in enumerate(seq):
                            mm = nc.tensor.matmul(d_[:, :], w_, r_,
                                                  start=(i_ == 0), stop=(i_ == 3))
                        mm.then_inc(s_pe, 1)    # A[k1c]
                    # transposes: slots pb4/pb5 (ch0) or pb0/pb1 (ch1),
                    # fp16-packed, n2c-major
                    twk = dm["tw0"] if ch == 0 else dm["tw1"]
                    if ch == 0 and it >= 1:
                        tensor.wait_ge(s_act, act_m(it - 1)["cy2"][1])
                    sbase = 4 if ch == 0 else 0
                    for n2c in range(4):
                        bank = sbase + n2c // 2
                        tensor.wait_ge(s_dve, twk[1])
                        last = None
                        for plane in range(2):
                            for k1c in range(2):
                                chunk = (n2c % 2) * 4 + plane * 2 + k1c
                                last = nc.tensor.transpose(
                                    ps16(bank, chunk), t2b(plane, k1c, n2c),
                                    ident[:, :])
                        last.then_inc(s_pe, 1)  # T[n2c]
                    # stage C: psC re01=pb4? NO: re01=pb4 conflicts slots...
                    # banks: re k2c01=pb6, re k2c23=pb7, im01=pb4, im23=pb5
                    # -- wait: slots were pb4/pb5; C writes pb4/pb5 only after
                    # cT evacuated them; C also writes pb6/pb7 (free).
                    cTk = am["cT0"] if ch == 0 else am["cT1"]
                    tensor.wait_ge(s_act, cTk[3])
                    for k2c in range(4):
                        pre_ = pb[6 + k2c // 2][:, (k2c % 2) * 256:
                                                (k2c % 2) * 256 + 256]
                        pim_ = pb[4 + k2c // 2][:, (k2c % 2) * 256:
                                                (k2c % 2) * 256 + 256]
                        seq = ([(pre_, w512c(n2c, k2c, 0), t2tc(n2c, 0))
                                for n2c in range(4)]
                               + [(pre_, w512c(n2c, k2c, 2), t2tc(n2c, 1))
                                  for n2c in range(4)])
                        for i_, (d_, w_, r_) in enumerate(seq):
                            nc.tensor.matmul(d_, w_, r_, start=(i_ == 0),
                                             stop=(i_ == 7))
                        seq = ([(pim_, w512c(n2c, k2c, 0), t2tc(n2c, 1))
                                for n2c in range(4)]
                               + [(pim_, w512c(n2c, k2c, 1), t2tc(n2c, 0))
                                  for n2c in range(4)])
                        for i_, (d_, w_, r_) in enumerate(seq):
                            mm = nc.tensor.matmul(d_, w_, r_, start=(i_ == 0),
                                                  stop=(i_ == 7))
                        mm.then_inc(s_pe, 1)    # C[k2c]

                # =================== inverse: A' for both outputs ============
                # o=0 -> pb0-3 (re01,re23,im01,im23); o=1 -> pb4-7
                for o in ([] if FWD_ONLY else range(2)):
                    if o == 0:
                        tensor.wait_ge(s_dve, dm["fdl1"][0])
                        tensor.wait_ge(s_act, am["cT1"][3])
                    else:
                        tensor.wait_ge(s_dve, dm["fdl1"][1])
                        tensor.wait_ge(s_act, am["cX1"][3])
                    base = 0 if o == 0 else 4
                    for n2c in range(4):
                        pre_ = pb[base + n2c // 2][:, (n2c % 2) * 256:
                                                   (n2c % 2) * 256 + 256]
                        pim_ = pb[base + 2 + n2c // 2][:, (n2c % 2) * 256:
                                                       (n2c % 2) * 256 + 256]
                        seq = ([(pre_, w512c(k2c, n2c, 0), ym_mv(o, k2c, 0))
                                for k2c in range(4)]
                               + [(pre_, w512c(k2c, n2c, 1), ym_mv(o, k2c, 1))
                                  for k2c in range(4)])
                        for i_, (d_, w_, r_) in enumerate(seq):
                            nc.tensor.matmul(d_, w_, r_, start=(i_ == 0),
                                             stop=(i_ == 7))
                        seq = ([(pim_, w512c(k2c, n2c, 0), ym_mv(o, k2c, 1))
                                for k2c in range(4)]
                               + [(pim_, w512c(k2c, n2c, 2), ym_mv(o, k2c, 0))
                                  for k2c in range(4)])
                        for i_, (d_, w_, r_) in enumerate(seq):
                            mm = nc.tensor.matmul(d_, w_, r_, start=(i_ == 0),
                                                  stop=(i_ == 7))
                        mm.then_inc(s_pe, 1)    # Ap/Ap2[n2c]

                # =================== inverse tail per output =================
                for o in ([] if FWD_ONLY else range(2)):
                    twpk = dm["twp"] if o == 0 else dm["twp2"]
                    cSk = am["cS"] if o == 0 else am["cS2"]
                    Cpk = pm["Cp"] if o == 0 else pm["Cp2"]
                    base = 0 if o == 0 else 4
                    # Tp transposes: slots pb[base]/pb[base+1]
                    for k1c in range(2):
                        for plane in range(2):
                            tensor.wait_ge(s_dve, twpk[plane])
                            last = None
                            for n2c in range(4):
                                chunk = plane * 4 + n2c
                                last = nc.tensor.transpose(
                                    ps16(base + k1c, chunk),
                                    s2b(o, plane, n2c, k1c), ident[:, :])
                            last.then_inc(s_pe, 1)   # Tp[(k1c,plane)]
                    # C': y planes -> pb[base..base+3]
                    tensor.wait_ge(s_act, cSk[3])
                    for n1c in range(2):
                        pre_ = pb[base + 2 * n1c]
                        pim_ = pb[base + 2 * n1c + 1]
                        seq = ([(pre_, w256c(k1c, n1c, 0), s2tc(o, k1c, 0))
                                for k1c in range(2)]
                               + [(pre_, w256c(k1c, n1c, 1), s2tc(o, k1c, 1))
                                  for k1c in range(2)])
                        for i_, (d_, w_, r_) in enumerate(seq):
                            nc.tensor.matmul(d_[:, :], w_, r_, start=(i_ == 0),
                                             stop=(i_ == 3))
                        seq = ([(pim_, w256c(k1c, n1c, 0), s2tc(o, k1c, 1))
                                for k1c in range(2)]
                               + [(pim_, w256c(k1c, n1c, 2), s2tc(o, k1c, 0))
                                  for k1c in range(2)])
                        for i_, (d_, w_, r_) in enumerate(seq):
                            mm = nc.tensor.matmul(d_[:, :], w_, r_,
                                                  start=(i_ == 0), stop=(i_ == 3))
                        mm.then_inc(s_pe, 1)    # Cp/Cp2[n1c]

        # ------------------------------------------------------------------
        @block.vector
        def _(vector):
            def cmul(dre, dim, are, aim, bre, bim, t1, t2_):
                nc.vector.tensor_mul(dre, are, bre)
                nc.vector.tensor_mul(t1, aim, bim)
                r3 = nc.vector.tensor_sub(dre, dre, t1)
                nc.vector.tensor_mul(dim, are, bim)
                nc.vector.tensor_mul(t2_, aim, bre)
                r6 = nc.vector.tensor_add(dim, dim, t2_)
                return r3, r6

            for it in range(NIT):
                pm, am = pe_m(it), act_m(it)
                for ch in range(2):
                    tAk = am["tA0"] if ch == 0 else am["tA1"]
                    cXk = am["cX0"] if ch == 0 else am["cX1"]
                    # forward twiddle: t2 = tA * TW   (all-SBUF fp16 2x)
                    vector.wait_ge(s_act, tAk[3])
                    r3, r6 = cmul(t2full(0), t2full(1), tAc(0), tAc(1),
                                  twc(0), twc(1), dtmp[:, 0:1024],
                                  dtmp[:, 1024:2048])
                    r3.then_inc(s_dve, 1)       # tw[0] (re plane)
                    r6.then_inc(s_dve, 1)       # tw[1] (im plane)
                    # FDL
                    vector.wait_ge(s_act, cXk[3])
                    xr = xspc(ch)[:, 0:1024]
                    xi = xspc(ch)[:, 1024:2048]
                    if ch == 0:
                        for o in range(2):
                            yr = ymc(o)[:, 0:1024]
                            yi = ymc(o)[:, 1024:2048]
                            t1 = dtmp[:, 0:1024]
                            nc.vector.tensor_mul(yr, xr, gc(o, 0, 0))
                            nc.vector.tensor_mul(t1, xi, gc(o, 0, 1))
                            nc.vector.tensor_sub(yr, yr, t1)
                            nc.vector.tensor_mul(yi, xr, gc(o, 0, 1))
                            nc.vector.tensor_mul(t1, xi, gc(o, 0, 0))
                            last = nc.vector.tensor_add(yi, yi, t1)
                        last.then_inc(s_dve, 1)     # fdl0
                    else:
                        for o in range(2):
                            yr = ymc(o)[:, 0:1024]
                            yi = ymc(o)[:, 1024:2048]
                            t1 = dtmp[:, 0:1024]
                            nc.vector.tensor_mul(t1, xr, gc(o, 1, 0))
                            nc.vector.tensor_add(yr, yr, t1)
                            nc.vector.tensor_mul(t1, xi, gc(o, 1, 1))
                            nc.vector.tensor_sub(yr, yr, t1)
                            nc.vector.tensor_mul(t1, xr, gc(o, 1, 1))
                            nc.vector.tensor_add(yi, yi, t1)
                            nc.vector.tensor_mul(t1, xi, gc(o, 1, 0))
                            nc.vector.tensor_add(yi, yi, t1).then_inc(s_dve, 1)
                            # fdl1[o]
                # inverse twiddle per output: s2 = sr * TWI (all-SBUF 2x)
                for o in ([] if FWD_ONLY else range(2)):
                    sAk = am["sA"] if o == 0 else am["sA2"]
                    vector.wait_ge(s_act, sAk[3])
                    r3, r6 = cmul(s2c(o, 0), s2c(o, 1), src_(o, 0), src_(o, 1),
                                  twic(0), twic(1), dtmp[:, 0:1024],
                                  dtmp[:, 1024:2048])
                    r3.then_inc(s_dve, 1)       # twp[0]
                    r6.then_inc(s_dve, 1)       # twp[1]

        # ------------------------------------------------------------------
        @block.scalar
        def _(scalar):
            for it in range(NIT):
                pm, am = pe_m(it), act_m(it)
                buf = it % 2
                for ch in range(2):
                    Ak = pm["A0"] if ch == 0 else pm["A1"]
                    Tk = pm["T0"] if ch == 0 else pm["T1"]
                    Ck = pm["C0"] if ch == 0 else pm["C1"]
                    # tA evac: psA banks -> tA fp16 (re: k1c-major | im)
                    # bank order: pb0=re k1c0, pb1=im k1c0, pb2=re k1c1, pb3=im
                    if ch == 1:
                        scalar.wait_ge(s_dve, dve_m(it)["tw0"][1])
                    dsts = [tA[:, 0:512], tA[:, 1024:1536],
                            tA[:, 512:1024], tA[:, 1536:2048]]
                    for i_, bank in enumerate([0, 1, 2, 3]):
                        scalar.wait_ge(s_pe, Ak[bank // 2])
                        nc.scalar.copy(dsts[i_], pb[bank][:, :]).then_inc(s_act, 1)
                    # cT evac: slots fp16 -> t2t (n2c-major)
                    # wait for BOTH n2c groups of the bank: no ACT read while
                    # the PE still writes the bank's other chunk region
                    sbase = 4 if ch == 0 else 0
                    for n2c in range(4):
                        scalar.wait_ge(s_pe, Tk[(n2c // 2) * 2 + 1])
                        bank = sbase + n2c // 2
                        src = pb[bank][:, (n2c % 2) * 256:(n2c % 2) * 256 + 256]
                        nc.scalar.copy(t2t[:, n2c * 512:n2c * 512 + 512],
                                       src.bitcast(HP)).then_inc(s_act, 1)
                    # cX evac: psC -> xsp fp16 (re|im, k2c-major)
                    # banks: re01=pb6 re23=pb7 im01=pb4 im23=pb5
                    pairs = [(6, 0), (4, 1024), (7, 512), (5, 1536)]
                    for i_, (bank, off) in enumerate(pairs):
                        scalar.wait_ge(s_pe, Ck[1] if i_ < 2 else Ck[3])
                        nc.scalar.copy(xspc(ch)[:, off:off + 512],
                                       pb[bank][:, :]).then_inc(s_act, 1)
                for o in ([] if FWD_ONLY else range(2)):
                    Apk = pm["Ap"] if o == 0 else pm["Ap2"]
                    Tpk = pm["Tp"] if o == 0 else pm["Tp2"]
                    Cpk = pm["Cp"] if o == 0 else pm["Cp2"]
                    base = 0 if o == 0 else 4
                    # sA evac: psA' -> sr fp16 (re n2c-major | im)
                    # banks: re01, re23, im01, im23
                    for i_, bank in enumerate([base, base + 1, base + 2,
                                               base + 3]):
                        scalar.wait_ge(s_pe, Apk[[1, 3, 1, 3][i_]])
                        off = [0, 512, 1024, 1536][i_]
                        nc.scalar.copy(sr[:, o * 2048 + off:o * 2048 + off + 512],
                                       pb[bank][:, :]).then_inc(s_act, 1)
                    # cS evac: Tp slots -> s2t fp16 per (k1c, plane)
                    for g_ in range(4):
                        k1c, plane = divmod(g_, 2)
                        scalar.wait_ge(s_pe, Tpk[k1c * 2 + 1])
                        bank = base + k1c
                        src = pb[bank][:, plane * 256:plane * 256 + 256]
                        nc.scalar.copy(
                            s2t[:, o * 2048 + (k1c * 2 + plane) * 512:
                                o * 2048 + (k1c * 2 + plane) * 512 + 512],
                            src.bitcast(HP)).then_inc(s_act, 1)
                    # cy: psC' -> ysb fp32
                    if o == 0 and it >= 2:
                        scalar.wait_ge(s_dma, out_after[it - 2])
                    for n1c in range(2):
                        scalar.wait_ge(s_pe, Cpk[n1c])
                        nc.scalar.copy(ysbc(buf, o, n1c, 0),
                                       pb[base + 2 * n1c][:, :])
                        nc.scalar.copy(ysbc(buf, o, n1c, 1),
                                       pb[base + 2 * n1c + 1][:, :]
                                       ).then_inc(s_act, 1)
    return nc


# ---------------------------- host side ----------------------------------

def make_device_tables(h):
    """h: (2,2,K_IR) float32 -> dict of DRAM table arrays (fp16)."""
    def dftm(n, sign):
        k = np.arange(n)
        return np.exp(sign * 2j * np.pi * np.outer(k, k) / n)
    W256 = dftm(N1, -1)
    W512 = dftm(N2, -1)
    k1 = np.arange(N1)
    n2 = np.arange(N2)
    TW = np.exp(-2j * np.pi * np.outer(k1, n2) / N)     # [k1, n2]
    TWI = np.exp(+2j * np.pi * np.outer(n2, k1) / N)    # [n2, k1] pure phase

    def chunks(pr, pc, planes):
        cols = []
        for P in planes:
            for a in range(pr):
                for b_ in range(pc):
                    cols.append(P[128 * a:128 * (a + 1),
                                  128 * b_:128 * (b_ + 1)])
        return np.ascontiguousarray(np.concatenate(cols, axis=1))

    w256 = chunks(2, 2, [W256.real, W256.imag, -W256.imag])
    w512 = chunks(4, 4, [W512.real, W512.imag, -W512.imag])

    # tw: [128, 2048]: plane-major, k1c-major inside: tw[p, pl*1024+k1c*512+n2]
    twp = np.zeros((128, 2048))
    for pl, P in enumerate([TW.real, TW.imag]):
        for k1c in range(2):
            twp[:, pl * 1024 + k1c * 512:pl * 1024 + (k1c + 1) * 512] = \
                P[k1c * 128:(k1c + 1) * 128, :]

    # twi: [128, 2048]: plane-major, n2c-major: twi[p, pl*1024+n2c*256+k1]
    twip = np.zeros((128, 2048))
    for pl, P in enumerate([TWI.real, TWI.imag]):
        for n2c in range(4):
            twip[:, pl * 1024 + n2c * 256:pl * 1024 + (n2c + 1) * 256] = \
                P[n2c * 128:(n2c + 1) * 128, :]

    # g: [128, 8192]: ((o*2+ch)*2+plane)*1024 + k2c*256 + k1 ; includes 1/N
    hp = np.zeros((2, 2, N), np.float64)
    hp[:, :, :K_IR] = h
    gt = np.zeros((128, 8192))
    for o in range(2):
        for ch in range(2):
            G = np.fft.fft(hp[o, ch]) / N
            Gm = G.reshape(N2, N1)          # [k2, k1]
            for pl, P in enumerate([Gm.real, Gm.imag]):
                for k2c in range(4):
                    off = ((o * 2 + ch) * 2 + pl) * 1024 + k2c * 256
                    gt[:, off:off + 256] = P[k2c * 128:(k2c + 1) * 128, :]

    return dict(w256=w256.astype(np.float16), w512=w512.astype(np.float16),
                tw=twp.astype(np.float16), twi=twip.astype(np.float16),
                g=gt.astype(np.float16))


_NC_CACHE = None


def make_in_maps(x, h):
    """Per-core input maps for the SPMD launch."""
    B = x.shape[0]
    assert B == 8 * NB
    xp = np.zeros((B, 2, PAD_LEN), np.float16)
    xp[:, :, PAD_PRE:PAD_PRE + T_SIG] = x.astype(np.float16)
    tabs = make_device_tables(h)
    in_maps = []
    for c in range(8):
        m = {"xp": xp[NB * c:NB * (c + 1)]}
        m.update(tabs)
        in_maps.append(m)
    return in_maps


def conv_device(x, h):
    """x: (B,2,T) fp32, h: (2,2,K_IR) fp32 -> y: (B,2,T) fp32 via 8 cores."""
    global _NC_CACHE
    from concourse.bass_utils import run_bass_kernel_spmd
    if _NC_CACHE is None:
        _NC_CACHE = build_nc()
    nc = _NC_CACHE
    in_maps = make_in_maps(x, h)
    res = run_bass_kernel_spmd(nc, in_maps, list(range(8)))
    y = np.concatenate([res.results[c]["y"] for c in range(8)], axis=0)
    return y


def kernel(**inputs):
    """Full FDN: build IR on host, FFT-convolve on 8 NeuronCores."""
    x = np.asarray(inputs["x"], np.float32)
    h = fdn_ir(np.asarray(inputs["b"]), np.asarray(inputs["c"]),
               np.asarray(inputs["U_raw"]), np.asarray(inputs["gamma_raw"]),
               np.asarray(inputs["delays"]))
    y = conv_device(x, h)
    return y.astype(np.float32)


# revision 11
# speedup vs baseline: 5.2321x; 1.6309x over previous
"""Self-contained Trainium2 kernel for nn_FDN_37211596653125 (v3).

kernel(**inputs) -> y (32,2,441000) float32.
Host: FDN impulse response (tiny 6x6 solves). Device (8 NeuronCores, raw
Bass): overlap-save FFT convolution, N=131072=256x512 Cooley-Tukey via PE
matmuls in fp16 (fp32 PSUM accumulate), batch-PAIRED complex FFTs
(z = x[2p] + i*x[2p+1] per channel; by linearity one complex FFT serves
two batches), twiddle/spectral products on VectorE in fp16 2x mode after
Act-engine PSUM evacuation, fp16 PE transposes into bitcast PSUM views.
"""
import sys
sys.path.insert(0, "/opt/trn_rl_repo")
import numpy as np
import concourse.bass as bass
import concourse.mybir as mybir
from concourse.masks import make_identity


SR = 44100
IR_LEN = 44100
T60 = 0.75
D = 6


def _expm(A):
    """Pade-13 scaling-and-squaring matrix exponential (float64)."""
    A = A.astype(np.float64)
    b = [64764752532480000.0, 32382376266240000.0, 7771770303897600.0,
         1187353796428800.0, 129060195264000.0, 10559470521600.0,
         670442572800.0, 33522128640.0, 1323241920.0, 40840800.0,
         960960.0, 16380.0, 182.0, 1.0]
    n = A.shape[0]
    nrm = np.linalg.norm(A, 1)
    theta13 = 5.371920351148152
    s = max(0, int(np.ceil(np.log2(max(nrm / theta13, 1e-300)))))
    if nrm <= theta13:
        s = 0
    A = A / (2.0 ** s)
    I = np.eye(n)
    A2 = A @ A
    A4 = A2 @ A2
    A6 = A2 @ A4
    U = A @ (A6 @ (b[13] * A6 + b[11] * A4 + b[9] * A2)
             + b[7] * A6 + b[5] * A4 + b[3] * A2 + b[1] * I)
    V = (A6 @ (b[12] * A6 + b[10] * A4 + b[8] * A2)
         + b[6] * A6 + b[4] * A4 + b[2] * A2 + b[0] * I)
    R = np.linalg.solve(V - U, V + U)
    for _ in range(s):
        R = R @ R
    return R


def fdn_ir(b, c, U_raw, gamma_raw, delays):
    """Build (2,2,IR_LEN) float32 FDN impulse response."""
    delays_f = delays.astype(np.float32)
    F_ = IR_LEN // 2 + 1
    gamma_max = np.float32(10.0) ** (np.float32(-60.0 / SR / T60 / 20.0)
                                     * delays_f)               # (D,)
    gamma = (1.0 / (1.0 + np.exp(-gamma_raw.astype(np.float32)))) * gamma_max  # (S,D)
    S = gamma.shape[0]
    pos = np.arange(F_, dtype=np.float32) * np.float32((S - 1) / (F_ - 1))
    lo = np.clip(np.floor(pos).astype(np.int32), 0, S - 2)
    frac = (pos - lo.astype(np.float32))[:, None]
    g = gamma[lo] * (1.0 - frac) + gamma[lo + 1] * frac         # (F,D) fp32

    tri = np.triu(U_raw.astype(np.float64), 1)
    U = _expm(tri - tri.T).astype(np.float32)                   # (D,D)

    A = U[None, :, :] * g[:, None, :]                           # (F,D,D)
    freqs = (np.arange(F_, dtype=np.float32) / np.float32(IR_LEN)
             * np.float32(2.0 * np.pi))
    phase = freqs[:, None] * delays_f[None, :]                  # fp32 like ref
    invD = np.exp(1j * phase.astype(np.float32)).astype(np.complex64)  # (F,D)
    eye = np.eye(D, dtype=np.complex64)
    M = invD[:, :, None] * eye[None] - A.astype(np.complex64)   # (F,D,D)
    b_c = np.broadcast_to(b.astype(np.complex64), (F_, D, 2))
    X = np.linalg.solve(M, b_c)                                 # (F,D,2)
    H = np.einsum('oi,fik->fok', c.astype(np.complex64), X)     # (F,2,2)
    h = np.fft.irfft(H.transpose(1, 2, 0), n=IR_LEN, axis=-1)   # (2,2,L)
    return h.astype(np.float32)


FP = mybir.dt.float32
HP = mybir.dt.float16
XP_NP = np.float16

N = 131072
N1, N2 = 256, 512
K_IR = 44100
L_HOP = N - K_IR + 1            # 86973
T_SIG = 441000
NBLK = 6
PAD_PRE = K_IR - 1              # 44099
PAD_LEN = (NBLK - 1) * L_HOP + N  # 565937
NB = 4                          # batches per core
NPAIR = 2                       # batch pairs per core
NIT = NPAIR * NBLK              # 12
FWD_ONLY = False
INV_ONLY = False                # debug: skip inverse half

# ---- per-slot semaphore increment schedules ------------------------------
# slot s: FWD(s) [s<NIT] interleaved with INV(s-1) [s>=1]; NSLOT = NIT+1
# Milestones count ONLY active groups (no dummy increments emitted).
NSLOT = NIT + 1
PE_ORDER = [("A0", 2, 1), ("Ap", 4, 0), ("T0", 4, 1), ("Ap2", 4, 0),
            ("C0", 4, 1), ("A1", 2, 1), ("Tp", 4, 0), ("Cp", 2, 0),
            ("T1", 4, 1), ("C1", 4, 1), ("Tp2", 4, 0), ("Cp2", 2, 0)]
DVE_ORDER = [("tw0", 2, 1), ("twp", 2, 0), ("twp2", 2, 0), ("tw1", 2, 1),
             ("fdl0", 1, 1), ("fdl1", 2, 1)]
ACT_ORDER = [("tA0", 4, 1), ("sA", 4, 0), ("cT0", 4, 1), ("sA2", 4, 0),
             ("cX0", 4, 1), ("tA1", 4, 1), ("cS", 4, 0), ("cy", 2, 0),
             ("cT1", 4, 1), ("cX1", 4, 1), ("cS2", 4, 0), ("cy2", 2, 0)]


def _mk(order):
    fwd_tot = sum(sz for _, sz, isf in order if isf)
    inv_tot = sum(sz for _, sz, isf in order if not isf)

    def m(s):
        c = 0
        for t in range(s):
            c += (fwd_tot if t < NIT else 0) + (inv_tot if t >= 1 else 0)
        d = {}
        for name, size, isf in order:
            active = (s < NIT) if isf else (s >= 1)
            if active:
                d[name] = [c + k for k in range(1, size + 1)]
                c += size
            else:
                d[name] = None
        return d
    return m


pe_m = _mk(PE_ORDER)
dve_m = _mk(DVE_ORDER)
act_m = _mk(ACT_ORDER)


def out_ranges(j):
    """DMA row ranges for valid region of block j (rows local to n1c chunk)."""
    valid = min(L_HOP, T_SIG - j * L_HOP)
    p0, p1 = PAD_PRE, PAD_PRE + valid
    res = []
    for n1c in range(2):
        base = 128 * n1c
        segs = []
        for R in range(base, base + 128):
            r0, r1 = 512 * R, 512 * R + 512
            s, e = max(r0, p0), min(r1, p1)
            if s >= e:
                continue
            segs.append((R, s - r0, e - r0, s))
        i = 0
        while i < len(segs):
            R, c0, c1, dst = segs[i]
            if c0 == 0 and c1 == 512:
                k = i
                while (k + 1 < len(segs) and segs[k + 1][1] == 0
                       and segs[k + 1][2] == 512):
                    k += 1
                res.append((n1c, segs[i][0] - base, segs[k][0] - base + 1,
                            0, 512, dst - p0 + j * L_HOP))
                i = k + 1
            else:
                res.append((n1c, R - base, R - base + 1, c0, c1,
                            dst - p0 + j * L_HOP))
                i += 1
    return res


def n_out_dmas(j):
    return 4 * len(out_ranges(j))   # 2 outputs x 2 planes (batches)


def build_nc():
    nc = bass.Bass(detect_race_conditions=False)
    xp_in = nc.declare_dram_parameter("xp", [NB, 2, PAD_LEN], HP, isOutput=False)
    w256_in = nc.declare_dram_parameter("w256", [128, 12 * 128], HP, isOutput=False)
    w512_in = nc.declare_dram_parameter("w512", [128, 48 * 128], HP, isOutput=False)
    tw_in = nc.declare_dram_parameter("tw", [128, 2048], HP, isOutput=False)
    twi_in = nc.declare_dram_parameter("twi", [128, 2048], HP, isOutput=False)
    g_in = nc.declare_dram_parameter("g", [128, 8192], HP, isOutput=False)
    y_out = nc.declare_dram_parameter("y", [NB, 2, T_SIG], FP, isOutput=True)

    NTAB = 5
    ld_ch0 = {}
    ld_all = {}
    out_after = {}
    v = 16 * NTAB
    for s in range(NSLOT):
        if s < NIT:
            v += 64; ld_ch0[s] = v
            v += 64; ld_all[s] = v
        if s >= 2:
            d = s - 2                       # outs of data-iteration d
            v += 16 * n_out_dmas(d % NBLK); out_after[d] = v
    v += 16 * n_out_dmas((NIT - 1) % NBLK); out_after[NIT - 1] = v

    from contextlib import ExitStack
    es = ExitStack()
    with es:
        w256 = es.enter_context(nc.sbuf_tensor([128, 12 * 128], HP))
        w512 = es.enter_context(nc.sbuf_tensor([128, 48 * 128], HP))
        tw = es.enter_context(nc.sbuf_tensor([128, 2048], HP))
        twi = es.enter_context(nc.sbuf_tensor([128, 2048], HP))
        gtab = es.enter_context(nc.sbuf_tensor([128, 8192], HP))
        ident = es.enter_context(nc.sbuf_tensor([128, 128], HP))
        m1 = es.enter_context(nc.sbuf_tensor([128, 2 * 8 * 512], HP))
        tA = es.enter_context(nc.sbuf_tensor([128, 2048], HP))
        t2 = es.enter_context(nc.sbuf_tensor([128, 2048], HP))
        t2t = es.enter_context(nc.sbuf_tensor([128, 2048], HP))
        xsp = es.enter_context(nc.sbuf_tensor([128, 2 * 2048], HP))
        ymat = es.enter_context(nc.sbuf_tensor([128, 2 * 2 * 2048], HP))
        sr = es.enter_context(nc.sbuf_tensor([128, 2 * 2048], HP))
        s2 = es.enter_context(nc.sbuf_tensor([128, 2 * 2048], HP))
        s2t = es.enter_context(nc.sbuf_tensor([128, 2 * 2048], HP))
        ysb = es.enter_context(nc.sbuf_tensor([128, 2 * 4096], FP))
        dtmp = es.enter_context(nc.sbuf_tensor([128, 2048], HP))
        pb = [es.enter_context(nc.psum_tensor(f"pb{i}", [128, N2], FP))
              for i in range(8)]
        s_dma = es.enter_context(nc.semaphore("s_dma"))
        s_pe = es.enter_context(nc.semaphore("s_pe"))
        s_dve = es.enter_context(nc.semaphore("s_dve"))
        s_act = es.enter_context(nc.semaphore("s_act"))
        s_gp = es.enter_context(nc.semaphore("s_gp"))
        block = es.enter_context(nc.Block())

        # ---------- layout helpers ----------
        def w256c(a, bq, plane):
            return w256[:, (plane * 4 + a * 2 + bq) * 128:
                        (plane * 4 + a * 2 + bq) * 128 + 128]

        def w512c(a, bq, plane):
            return w512[:, (plane * 16 + a * 4 + bq) * 128:
                        (plane * 16 + a * 4 + bq) * 128 + 128]

        def m1c(buf, ch, plane, n1c):
            o = buf * 4096 + ((ch * 2 + plane) * 2 + n1c) * 512
            return m1[:, o:o + 512]

        def tAc(plane):
            return tA[:, plane * 1024:plane * 1024 + 1024]

        def t2b(plane, k1c, n2c):
            o = plane * 1024 + k1c * 512 + n2c * 128
            return t2[:, o:o + 128]

        def t2full(plane):
            return t2[:, plane * 1024:plane * 1024 + 1024]

        def t2tc(n2c, plane):
            o = n2c * 512 + plane * 256
            return t2t[:, o:o + 256]

        def xspc(ch):
            return xsp[:, ch * 2048:ch * 2048 + 2048]

        def ymc(buf, o):                    # [128,2048] per (buf, output)
            o_ = (buf * 2 + o) * 2048
            return ymat[:, o_:o_ + 2048]

        def ym_mv(buf, o, k2c, plane):      # [128,256] A' moving operand
            o_ = (buf * 2 + o) * 2048 + plane * 1024 + k2c * 256
            return ymat[:, o_:o_ + 256]

        def src_(o, plane):
            return sr[:, o * 2048 + plane * 1024:o * 2048 + plane * 1024 + 1024]

        def s2c(o, plane):
            return s2[:, o * 2048 + plane * 1024:o * 2048 + plane * 1024 + 1024]

        def s2b(o, plane, n2c, k1c):
            o_ = o * 2048 + plane * 1024 + n2c * 256 + k1c * 128
            return s2[:, o_:o_ + 128]

        def s2tc(o, k1c, plane):
            o_ = o * 2048 + (k1c * 2 + plane) * 512
            return s2t[:, o_:o_ + 512]

        def gc(o, ch, plane):
            o_ = ((o * 2 + ch) * 2 + plane) * 1024
            return gtab[:, o_:o_ + 1024]

        def twc(plane):
            return tw[:, plane * 1024:plane * 1024 + 1024]

        def twic(plane):
            return twi[:, plane * 1024:plane * 1024 + 1024]

        def ysbc(buf, o, n1c, plane):
            o_ = buf * 4096 + o * 2048 + (n1c * 2 + plane) * 512
            return ysb[:, o_:o_ + 512]

        def ps16(bank, chunk):
            return pb[bank][:, chunk * 64:chunk * 64 + 64].bitcast(HP)

        @block.gpsimd
        def _(gpsimd):
            make_identity(nc, ident[:, :])
            gpsimd.sem_inc(s_gp, 1)

        # ------------------------------------------------------------------
        @block.sync
        def _(sync):
            sync.dma_start(w256[:, :], w256_in[:, :]).then_inc(s_dma, 16)
            sync.dma_start(w512[:, :], w512_in[:, :]).then_inc(s_dma, 16)
            sync.dma_start(tw[:, :], tw_in[:, :]).then_inc(s_dma, 16)
            sync.dma_start(twi[:, :], twi_in[:, :]).then_inc(s_dma, 16)
            sync.dma_start(gtab[:, :], g_in[:, :]).then_inc(s_dma, 16)

            def emit_outs(d, o):            # outs of data-iteration d
                p_, j = divmod(d, NBLK)
                buf = (d + 1) % 2           # ysb written in slot d+1
                am = act_m(d + 1)
                sync.wait_ge(s_act, am["cy"][1] if o == 0 else am["cy2"][1])
                for plane in range(2):
                    for (n1c, rlo, rhi, clo, chi, doff) in out_ranges(j):
                        src = ysbc(buf, o, n1c, plane)[rlo:rhi, clo:chi]
                        cnt = (rhi - rlo) * (chi - clo)
                        dst = y_out[2 * p_ + plane, o, doff:doff + cnt]
                        dst = dst.rearrange("(p f) -> p f", f=chi - clo)
                        sync.dma_start(dst, src).then_inc(s_dma, 16)

            for s in range(NSLOT):
                if s < NIT:
                    p_, j = divmod(s, NBLK)
                    buf = s % 2
                    if s >= 2:
                        sync.wait_ge(s_pe, pe_m(s - 2)["A1"][1])
                    for ch in range(2):
                        for plane in range(2):
                            srcx = xp_in[2 * p_ + plane, ch,
                                         j * L_HOP: j * L_HOP + N]
                            srcx = srcx.rearrange("(p f) -> p f", f=N2)
                            for n1c in range(2):
                                sync.dma_start(
                                    m1c(buf, ch, plane, n1c),
                                    srcx[128 * n1c:128 * (n1c + 1), :],
                                ).then_inc(s_dma, 16)
                if s >= 2:
                    emit_outs(s - 2, 0)
                    emit_outs(s - 2, 1)
            emit_outs(NIT - 1, 0)
            emit_outs(NIT - 1, 1)

        # ------------------------------------------------------------------
        @block.tensor
        def _(tensor):
            tensor.wait_ge(s_gp, 1)
            tensor.wait_ge(s_dma, 16 * NTAB)
            for s in range(NSLOT):
                pm, dm, am = pe_m(s), dve_m(s), act_m(s)
                fwd_on = s < NIT
                inv_on = s >= 1
                buf = s % 2
                ybuf = (s - 1) % 2          # ymat buffer read by inverse

                def fwd_A(ch, abase, Ak):
                    for k1c in range(2):
                        pre, pim = pb[abase + 2 * k1c], pb[abase + 2 * k1c + 1]
                        seq = ([(pre, w256c(n1c, k1c, 0), m1c(buf, ch, 0, n1c))
                                for n1c in range(2)]
                               + [(pre, w256c(n1c, k1c, 2), m1c(buf, ch, 1, n1c))
                                  for n1c in range(2)])
                        for i_, (d_, w_, r_) in enumerate(seq):
                            nc.tensor.matmul(d_[:, :], w_, r_, start=(i_ == 0),
                                             stop=(i_ == 3))
                        seq = ([(pim, w256c(n1c, k1c, 1), m1c(buf, ch, 0, n1c))
                                for n1c in range(2)]
                               + [(pim, w256c(n1c, k1c, 0), m1c(buf, ch, 1, n1c))
                                  for n1c in range(2)])
                        for i_, (d_, w_, r_) in enumerate(seq):
                            mm = nc.tensor.matmul(d_[:, :], w_, r_,
                                                  start=(i_ == 0), stop=(i_ == 3))
                        mm.then_inc(s_pe, 1)

                def fwd_T(sbase, twk):
                    for n2c in range(4):
                        bank = sbase + n2c // 2
                        tensor.wait_ge(s_dve, twk[1])
                        last = None
                        for plane in range(2):
                            for k1c in range(2):
                                chunk = (n2c % 2) * 4 + plane * 2 + k1c
                                last = nc.tensor.transpose(
                                    ps16(bank, chunk), t2b(plane, k1c, n2c),
                                    ident[:, :])
                        last.then_inc(s_pe, 1)

                def fwd_C(reb, imb):
                    for k2c in range(4):
                        pre_ = pb[reb + k2c // 2][:, (k2c % 2) * 256:
                                                  (k2c % 2) * 256 + 256]
                        pim_ = pb[imb + k2c // 2][:, (k2c % 2) * 256:
                                                  (k2c % 2) * 256 + 256]
                        seq = ([(pre_, w512c(n2c, k2c, 0), t2tc(n2c, 0))
                                for n2c in range(4)]
                               + [(pre_, w512c(n2c, k2c, 2), t2tc(n2c, 1))
                                  for n2c in range(4)])
                        for i_, (d_, w_, r_) in enumerate(seq):
                            nc.tensor.matmul(d_, w_, r_, start=(i_ == 0),
                                             stop=(i_ == 7))
                        seq = ([(pim_, w512c(n2c, k2c, 0), t2tc(n2c, 1))
                                for n2c in range(4)]
                               + [(pim_, w512c(n2c, k2c, 1), t2tc(n2c, 0))
                                  for n2c in range(4)])
                        for i_, (d_, w_, r_) in enumerate(seq):
                            mm = nc.tensor.matmul(d_, w_, r_, start=(i_ == 0),
                                                  stop=(i_ == 7))
                        mm.then_inc(s_pe, 1)

                def inv_A(o):
                    for n2c in range(4):
                        pre_ = pb[4 + n2c // 2][:, (n2c % 2) * 256:
                                                (n2c % 2) * 256 + 256]
                        pim_ = pb[6 + n2c // 2][:, (n2c % 2) * 256:
                                                (n2c % 2) * 256 + 256]
                        seq = ([(pre_, w512c(k2c, n2c, 0), ym_mv(ybuf, o, k2c, 0))
                                for k2c in range(4)]
                               + [(pre_, w512c(k2c, n2c, 1), ym_mv(ybuf, o, k2c, 1))
                                  for k2c in range(4)])
                        for i_, (d_, w_, r_) in enumerate(seq):
                            nc.tensor.matmul(d_, w_, r_, start=(i_ == 0),
                                             stop=(i_ == 7))
                        seq = ([(pim_, w512c(k2c, n2c, 0), ym_mv(ybuf, o, k2c, 1))
                                for k2c in range(4)]
                               + [(pim_, w512c(k2c, n2c, 2), ym_mv(ybuf, o, k2c, 0))
                                  for k2c in range(4)])
                        for i_, (d_, w_, r_) in enumerate(seq):
                            mm = nc.tensor.matmul(d_, w_, r_, start=(i_ == 0),
                                                  stop=(i_ == 7))
                        mm.then_inc(s_pe, 1)

                def inv_T(o, twpk):
                    for k1c in range(2):
                        for plane in range(2):
                            tensor.wait_ge(s_dve, twpk[plane])
                            last = None
                            for n2c in range(4):
                                chunk = plane * 4 + n2c
                                last = nc.tensor.transpose(
                                    ps16(k1c, chunk),
                                    s2b(o, plane, n2c, k1c), ident[:, :])
                            last.then_inc(s_pe, 1)

                def inv_C(o):
                    for n1c in range(2):
                        pre_ = pb[2 * n1c]
                        pim_ = pb[2 * n1c + 1]
                        seq = ([(pre_, w256c(k1c, n1c, 0), s2tc(o, k1c, 0))
                                for k1c in range(2)]
                               + [(pre_, w256c(k1c, n1c, 1), s2tc(o, k1c, 1))
                                  for k1c in range(2)])
                        for i_, (d_, w_, r_) in enumerate(seq):
                            nc.tensor.matmul(d_[:, :], w_, r_, start=(i_ == 0),
                                             stop=(i_ == 3))
                        seq = ([(pim_, w256c(k1c, n1c, 0), s2tc(o, k1c, 1))
                                for k1c in range(2)]
                               + [(pim_, w256c(k1c, n1c, 2), s2tc(o, k1c, 0))
                                  for k1c in range(2)])
                        for i_, (d_, w_, r_) in enumerate(seq):
                            mm = nc.tensor.matmul(d_[:, :], w_, r_,
                                                  start=(i_ == 0), stop=(i_ == 3))
                        mm.then_inc(s_pe, 1)

                # ---- slot program (see PE_PER comment for order) ----
                if fwd_on:                                  # A0
                    tensor.wait_ge(s_dma, ld_ch0[s])
                    if s >= 2:
                        tensor.wait_ge(s_act, act_m(s - 1)["cy2"][1])
                    fwd_A(0, 0, None)
                if inv_on:                                  # Ap (o=0)
                    tensor.wait_ge(s_dve, dve_m(s - 1)["fdl1"][0])
                    tensor.wait_ge(s_act, act_m(s - 1)["cX1"][3])
                    inv_A(0)
                if fwd_on:                                  # T0
                    fwd_T(0, dm["tw0"])
                if inv_on:                                  # Ap2 (o=1)
                    tensor.wait_ge(s_dve, dve_m(s - 1)["fdl1"][1])
                    tensor.wait_ge(s_act, am["sA"][3])
                    inv_A(1)
                if fwd_on:                                  # C0
                    tensor.wait_ge(s_act, am["cT0"][3])
                    fwd_C(0, 2)
                if fwd_on:                                  # A1
                    tensor.wait_ge(s_dma, ld_all[s])
                    if s >= 1:
                        tensor.wait_ge(s_act, am["sA2"][3])
                    fwd_A(1, 4, None)
                if inv_on:                                  # Tp (o=0)
                    if s < NIT:
                        tensor.wait_ge(s_act, am["cX0"][3])
                    else:
                        tensor.wait_ge(s_act, act_m(s - 1)["cy2"][1])
                    inv_T(0, dm["twp"])
                if inv_on:                                  # Cp (o=0)
                    tensor.wait_ge(s_act, am["cS"][3])
                    inv_C(0)
                if fwd_on:                                  # T1
                    fwd_T(4, dm["tw1"])
                if fwd_on:                                  # C1
                    tensor.wait_ge(s_act, am["cT1"][3])
                    fwd_C(4, 6)
                if inv_on:                                  # Tp2 (o=1)
                    tensor.wait_ge(s_act, am["cy"][1])
                    inv_T(1, dm["twp2"])
                if inv_on:                                  # Cp2 (o=1)
                    tensor.wait_ge(s_act, am["cS2"][3])
                    inv_C(1)

        # ------------------------------------------------------------------
        @block.vector
        def _(vector):
            def cmul(dre, dim, are, aim, bre, bim, t1, t2_):
                nc.vector.tensor_mul(dre, are, bre)
                nc.vector.tensor_mul(t1, aim, bim)
                r3 = nc.vector.tensor_sub(dre, dre, t1)
                nc.vector.tensor_mul(dim, are, bim)
                nc.vector.tensor_mul(t2_, aim, bre)
                r6 = nc.vector.tensor_add(dim, dim, t2_)
                return r3, r6

            for s in range(NSLOT):
                am = act_m(s)
                fwd_on = s < NIT
                inv_on = s >= 1
                buf = s % 2
                if fwd_on:                                  # tw0
                    vector.wait_ge(s_act, am["tA0"][3])
                    r3, r6 = cmul(t2full(0), t2full(1), tAc(0), tAc(1),
                                  twc(0), twc(1), dtmp[:, 0:1024],
                                  dtmp[:, 1024:2048])
                    r3.then_inc(s_dve, 1)
                    r6.then_inc(s_dve, 1)
                if inv_on:                                  # twp
                    vector.wait_ge(s_act, am["sA"][3])
                    r3, r6 = cmul(s2c(0, 0), s2c(0, 1), src_(0, 0), src_(0, 1),
                                  twic(0), twic(1), dtmp[:, 0:1024],
                                  dtmp[:, 1024:2048])
                    r3.then_inc(s_dve, 1)
                    r6.then_inc(s_dve, 1)
                if inv_on:                                  # twp2
                    vector.wait_ge(s_act, am["sA2"][3])
                    r3, r6 = cmul(s2c(1, 0), s2c(1, 1), src_(1, 0), src_(1, 1),
                                  twic(0), twic(1), dtmp[:, 0:1024],
                                  dtmp[:, 1024:2048])
                    r3.then_inc(s_dve, 1)
                    r6.then_inc(s_dve, 1)
                if fwd_on:                                  # tw1
                    vector.wait_ge(s_act, am["tA1"][3])
                    r3, r6 = cmul(t2full(0), t2full(1), tAc(0), tAc(1),
                                  twc(0), twc(1), dtmp[:, 0:1024],
                                  dtmp[:, 1024:2048])
                    r3.then_inc(s_dve, 1)
                    r6.then_inc(s_dve, 1)
                if fwd_on:                                  # fdl0 (ch0)
                    vector.wait_ge(s_act, am["cX0"][3])
                    xr = xspc(0)[:, 0:1024]
                    xi = xspc(0)[:, 1024:2048]
                    for o in range(2):
                        yr = ymc(buf, o)[:, 0:1024]
                        yi = ymc(buf, o)[:, 1024:2048]
                        t1 = dtmp[:, 0:1024]
                        nc.vector.tensor_mul(yr, xr, gc(o, 0, 0))
                        nc.vector.tensor_mul(t1, xi, gc(o, 0, 1))
                        nc.vector.tensor_sub(yr, yr, t1)
                        nc.vector.tensor_mul(yi, xr, gc(o, 0, 1))
                        nc.vector.tensor_mul(t1, xi, gc(o, 0, 0))
                        last = nc.vector.tensor_add(yi, yi, t1)
                    last.then_inc(s_dve, 1)
                if fwd_on:                                  # fdl1 (ch1, by output)
                    vector.wait_ge(s_act, am["cX1"][3])
                    xr = xspc(1)[:, 0:1024]
                    xi = xspc(1)[:, 1024:2048]
                    for o in range(2):
                        yr = ymc(buf, o)[:, 0:1024]
                        yi = ymc(buf, o)[:, 1024:2048]
                        t1 = dtmp[:, 0:1024]
                        nc.vector.tensor_mul(t1, xr, gc(o, 1, 0))
                        nc.vector.tensor_add(yr, yr, t1)
                        nc.vector.tensor_mul(t1, xi, gc(o, 1, 1))
                        nc.vector.tensor_sub(yr, yr, t1)
                        nc.vector.tensor_mul(t1, xr, gc(o, 1, 1))
                        nc.vector.tensor_add(yi, yi, t1)
                        nc.vector.tensor_mul(t1, xi, gc(o, 1, 0))
                        nc.vector.tensor_add(yi, yi, t1).then_inc(s_dve, 1)

        # ------------------------------------------------------------------
        @block.scalar
        def _(scalar):
            for s in range(NSLOT):
                pm, dm = pe_m(s), dve_m(s)
                fwd_on = s < NIT
                inv_on = s >= 1
                buf = s % 2
                tA_dst = [tA[:, 0:512], tA[:, 1024:1536],
                          tA[:, 512:1024], tA[:, 1536:2048]]
                sr_off = [0, 1024, 512, 1536]

                if fwd_on:                                  # tA0
                    for i_, bank in enumerate([0, 1, 2, 3]):
                        scalar.wait_ge(s_pe, pm["A0"][bank // 2])
                        nc.scalar.copy(tA_dst[i_], pb[bank][:, :]
                                       ).then_inc(s_act, 1)
                if inv_on:                                  # sA (o=0)
                    for i_, bank in enumerate([4, 6, 5, 7]):
                        scalar.wait_ge(s_pe, pm["Ap"][[1, 1, 3, 3][i_]])
                        off = sr_off[i_]
                        nc.scalar.copy(sr[:, off:off + 512], pb[bank][:, :]
                                       ).then_inc(s_act, 1)
                if fwd_on:                                  # cT0
                    for n2c in range(4):
                        scalar.wait_ge(s_pe, pm["T0"][(n2c // 2) * 2 + 1])
                        src = pb[0 + n2c // 2][:, (n2c % 2) * 256:
                                               (n2c % 2) * 256 + 256]
                        nc.scalar.copy(t2t[:, n2c * 512:n2c * 512 + 512],
                                       src.bitcast(HP)).then_inc(s_act, 1)
                if inv_on:                                  # sA2 (o=1)
                    for i_, bank in enumerate([4, 6, 5, 7]):
                        scalar.wait_ge(s_pe, pm["Ap2"][[1, 1, 3, 3][i_]])
                        off = 2048 + sr_off[i_]
                        nc.scalar.copy(sr[:, off:off + 512], pb[bank][:, :]
                                       ).then_inc(s_act, 1)
                if fwd_on:                                  # cX0
                    dsts = [0, 1024, 512, 1536]
                    for i_, bank in enumerate([0, 2, 1, 3]):
                        scalar.wait_ge(s_pe, pm["C0"][[1, 1, 3, 3][i_]])
                        off = dsts[i_]
                        nc.scalar.copy(xspc(0)[:, off:off + 512],
                                       pb[bank][:, :]).then_inc(s_act, 1)
                if fwd_on:                                  # tA1
                    scalar.wait_ge(s_dve, dm["tw0"][1])
                    for i_, bank in enumerate([4, 5, 6, 7]):
                        scalar.wait_ge(s_pe, pm["A1"][bank // 2 - 2])
                        nc.scalar.copy(tA_dst[i_], pb[bank][:, :]
                                       ).then_inc(s_act, 1)
                if inv_on:                                  # cS (o=0)
                    for g_ in range(4):
                        k1c, plane = divmod(g_, 2)
                        scalar.wait_ge(s_pe, pm["Tp"][k1c * 2 + 1])
                        src = pb[k1c][:, plane * 256:plane * 256 + 256]
                        nc.scalar.copy(
                            s2t[:, (k1c * 2 + plane) * 512:
                                (k1c * 2 + plane) * 512 + 512],
                            src.bitcast(HP)).then_inc(s_act, 1)
                if inv_on:                                  # cy (o=0)
                    if s >= 3:
                        scalar.wait_ge(s_dma, out_after[s - 3])
                    for n1c in range(2):
                        scalar.wait_ge(s_pe, pm["Cp"][n1c])
                        nc.scalar.copy(ysbc(buf, 0, n1c, 0),
                                       pb[2 * n1c][:, :])
                        nc.scalar.copy(ysbc(buf, 0, n1c, 1),
                                       pb[2 * n1c + 1][:, :]
                                       ).then_inc(s_act, 1)
                if fwd_on:                                  # cT1
                    for n2c in range(4):
                        scalar.wait_ge(s_pe, pm["T1"][(n2c // 2) * 2 + 1])
                        src = pb[4 + n2c // 2][:, (n2c % 2) * 256:
                                               (n2c % 2) * 256 + 256]
                        nc.scalar.copy(t2t[:, n2c * 512:n2c * 512 + 512],
                                       src.bitcast(HP)).then_inc(s_act, 1)
                if fwd_on:                                  # cX1
                    dsts = [0, 1024, 512, 1536]
                    for i_, bank in enumerate([4, 6, 5, 7]):
                        scalar.wait_ge(s_pe, pm["C1"][[1, 1, 3, 3][i_]])
                        off = dsts[i_]
                        nc.scalar.copy(xspc(1)[:, off:off + 512],
                                       pb[bank][:, :]).then_inc(s_act, 1)
                if inv_on:                                  # cS2 (o=1)
                    for g_ in range(4):
                        k1c, plane = divmod(g_, 2)
                        scalar.wait_ge(s_pe, pm["Tp2"][k1c * 2 + 1])
                        src = pb[k1c][:, plane * 256:plane * 256 + 256]
                        nc.scalar.copy(
                            s2t[:, 2048 + (k1c * 2 + plane) * 512:
                                2048 + (k1c * 2 + plane) * 512 + 512],
                            src.bitcast(HP)).then_inc(s_act, 1)
                if inv_on:                                  # cy2 (o=1)
                    for n1c in range(2):
                        scalar.wait_ge(s_pe, pm["Cp2"][n1c])
                        nc.scalar.copy(ysbc(buf, 1, n1c, 0),
                                       pb[2 * n1c][:, :])
                        nc.scalar.copy(ysbc(buf, 1, n1c, 1),
                                       pb[2 * n1c + 1][:, :]
                                       ).then_inc(s_act, 1)
    return nc


# ---------------------------- host side ----------------------------------

def make_device_tables(h):
    """h: (2,2,K_IR) float32 -> dict of DRAM table arrays (fp16)."""
    def dftm(n, sign):
        k = np.arange(n)
        return np.exp(sign * 2j * np.pi * np.outer(k, k) / n)
    W256 = dftm(N1, -1)
    W512 = dftm(N2, -1)
    k1 = np.arange(N1)
    n2 = np.arange(N2)
    TW = np.exp(-2j * np.pi * np.outer(k1, n2) / N)     # [k1, n2]
    TWI = np.exp(+2j * np.pi * np.outer(n2, k1) / N)    # [n2, k1] pure phase

    def chunks(pr, pc, planes):
        cols = []
        for P in planes:
            for a in range(pr):
                for b_ in range(pc):
                    cols.append(P[128 * a:128 * (a + 1),
                                  128 * b_:128 * (b_ + 1)])
        return np.ascontiguousarray(np.concatenate(cols, axis=1))

    w256 = chunks(2, 2, [W256.real, W256.imag, -W256.imag])
    w512 = chunks(4, 4, [W512.real, W512.imag, -W512.imag])

    # tw: [128, 2048]: plane-major, k1c-major inside: tw[p, pl*1024+k1c*512+n2]
    twp = np.zeros((128, 2048))
    for pl, P in enumerate([TW.real, TW.imag]):
        for k1c in range(2):
            twp[:, pl * 1024 + k1c * 512:pl * 1024 + (k1c + 1) * 512] = \
                P[k1c * 128:(k1c + 1) * 128, :]

    # twi: [128, 2048]: plane-major, n2c-major: twi[p, pl*1024+n2c*256+k1]
    twip = np.zeros((128, 2048))
    for pl, P in enumerate([TWI.real, TWI.imag]):
        for n2c in range(4):
            twip[:, pl * 1024 + n2c * 256:pl * 1024 + (n2c + 1) * 256] = \
                P[n2c * 128:(n2c + 1) * 128, :]

    # g: [128, 8192]: ((o*2+ch)*2+plane)*1024 + k2c*256 + k1 ; includes 1/N
    hp = np.zeros((2, 2, N), np.float64)
    hp[:, :, :K_IR] = h
    gt = np.zeros((128, 8192))
    for o in range(2):
        for ch in range(2):
            G = np.fft.fft(hp[o, ch]) / N
            Gm = G.reshape(N2, N1)          # [k2, k1]
            for pl, P in enumerate([Gm.real, Gm.imag]):
                for k2c in range(4):
                    off = ((o * 2 + ch) * 2 + pl) * 1024 + k2c * 256
                    gt[:, off:off + 256] = P[k2c * 128:(k2c + 1) * 128, :]

    return dict(w256=w256.astype(np.float16), w512=w512.astype(np.float16),
                tw=twp.astype(np.float16), twi=twip.astype(np.float16),
                g=gt.astype(np.float16))


_NC_CACHE = None


def make_in_maps(x, h):
    """Per-core input maps for the SPMD launch."""
    B = x.shape[0]
    assert B == 8 * NB
    xp = np.zeros((B, 2, PAD_LEN), np.float16)
    xp[:, :, PAD_PRE:PAD_PRE + T_SIG] = x.astype(np.float16)
    tabs = make_device_tables(h)
    in_maps = []
    for c in range(8):
        m = {"xp": xp[NB * c:NB * (c + 1)]}
        m.update(tabs)
        in_maps.append(m)
    return in_maps


def conv_device(x, h):
    """x: (B,2,T) fp32, h: (2,2,K_IR) fp32 -> y: (B,2,T) fp32 via 8 cores."""
    global _NC_CACHE
    from concourse.bass_utils import run_bass_kernel_spmd
    if _NC_CACHE is None:
        _NC_CACHE = build_nc()
    nc = _NC_CACHE
    in_maps = make_in_maps(x, h)
    res = run_bass_kernel_spmd(nc, in_maps, list(range(8)))
    y = np.concatenate([res.results[c]["y"] for c in range(8)], axis=0)
    return y


def kernel(**inputs):
    """Full FDN: build IR on host, FFT-convolve on 8 NeuronCores."""
    x = np.asarray(inputs["x"], np.float32)
    h = fdn_ir(np.asarray(inputs["b"]), np.asarray(inputs["c"]),
               np.asarray(inputs["U_raw"]), np.asarray(inputs["gamma_raw"]),
               np.asarray(inputs["delays"]))
    y = conv_device(x, h)
    return y.astype(np.float32)
